# revision 27
# baseline (speedup 1.0000x reference)
"""Trainium2 Bass kernel for an ALBERT-style seq2seq block (self-attn + cross-attn).

Sharding: 8 cores = (batch b in 0..3) x (decoder-row half in 0..1); zero
inter-core communication. Each core computes its 512 decoder rows.

v2 design vs baseline:
- All activations/weights in bf16; k/v projections in fp8(e4m3) with
  DoubleRow perf-mode matmuls (2 k-subtiles per pass, 0.5 cyc/col).
- v computed directly in [keys, vdim] layout (stationary = activation
  chunk), eliminating all PE transposes.
- k bias dropped (softmax shift-invariant), v bias folded into the
  out-projection bias on the host (bd_eff = bd + bv @ Wd.T).
- PSUM evictions and softmax-denominator copies moved off the Scalar
  engine (exp is the Scalar bottleneck) to GpSimd/Vector.
- Leaner LN tail (Rsqrt fused); held fillers drain BEFORE the broadcast
  matmuls so the PE queue is never head-of-line blocked on LN stats.
"""

import sys

sys.path.insert(0, "/opt/trn_rl_repo")

import numpy as np
import ml_dtypes

import concourse.bacc as bacc
import concourse.mybir as mybir
from concourse.bass_utils import run_bass_kernel_spmd
from concourse.tile import TileContext

F32 = mybir.dt.float32
F32R = mybir.dt.float32r
BF16 = mybir.dt.bfloat16
F8 = mybir.dt.float8e4
AF = mybir.ActivationFunctionType
ALU = mybir.AluOpType
DR = mybir.MatmulPerfMode.DoubleRow

P = 128          # partitions
H = 1024         # hidden
NT = H // P      # 8 tiles over hidden
NH = 16          # heads
D = 64           # head dim
T = 1024         # sequence length (encoder and decoder)
R = 512          # decoder rows per core
B = 4
EPS = 1e-12
WS = 32.0        # fp8 weight pre-scale (host); descaled at PSUM eviction
INV = 1.0 / WS


def build_kernel():
    nc = bacc.Bacc("TRN2", num_devices=8)

    dec8_d = nc.declare_dram_parameter("dec8", [P, NT, T], F8, isOutput=False)
    enc8_d = nc.declare_dram_parameter("enc8", [P, NT, T], F8, isOutput=False)
    dqb_d = nc.declare_dram_parameter("dqb", [P, NT, R], BF16, isOutput=False)
    wq_d = nc.declare_dram_parameter("wq", [NT, P, NT, P], BF16, isOutput=False)
    wk_d = nc.declare_dram_parameter("wk", [P, NT, NT, P], F8, isOutput=False)
    wv_d = nc.declare_dram_parameter("wv", [P, NT, H], F8, isOutput=False)
    wd_d = nc.declare_dram_parameter("wd", [NT, P, NT, P], F8, isOutput=False)
    bq_d = nc.declare_dram_parameter("bq", [P, NT, 1], F32, isOutput=False)
    bde_d = nc.declare_dram_parameter("bde", [P, NT, 1], F32, isOutput=False)
    lng_d = nc.declare_dram_parameter("lng", [P, NT, 1], F32, isOutput=False)
    lnb_d = nc.declare_dram_parameter("lnb", [P, NT, 1], F32, isOutput=False)
    mt_d = nc.declare_dram_parameter("mt", [P, NT, 1], F32, isOutput=False)
    ms_d = nc.declare_dram_parameter("ms", [P, NT, 1], F32, isOutput=False)
    onesrr_d = nc.declare_dram_parameter("onesr", [1, P], F32, isOutput=False)
    out_d = nc.declare_dram_parameter("out", [P, NT, R], F32, isOutput=True)

    with TileContext(nc) as tc:
        with tc.tile_pool(name="base", bufs=1) as base:
            x8 = base.tile([P, NT, T], F8, tag="x8")
            x8e = base.tile([P, NT, T], F8, tag="x8e")
            dqb = base.tile([P, NT, R], BF16, tag="dqb")
            wk = base.tile([P, NT, NT, P], F8, tag="wk")
            wv = base.tile([P, NT, H], F8, tag="wv")
            kT = base.tile([P, NT, T], BF16, tag="kT")
            vi = base.tile([P, NT, NH, P], BF16, tag="vi")
            q1b = [base.tile([P, R], BF16, tag=f"q1b{o}", name=f"q1b{o}") for o in range(NT)]
            slfb = [base.tile([P, R], BF16, tag=f"slfb{o}", name=f"slfb{o}") for o in range(NT)]
            ctxn = base.tile([P, NT, R], F8, tag="ctxn")
            resT = base.tile([P, NT, R], BF16, tag="resT")
            partialA = base.tile([P, NT, R], F32, tag="partialA")
            wdh = base.tile([P, NT, 4, P], F8, tag="wdh")
            bqc = base.tile([P, NT, 1], F32, tag="bqc")
            bdec = base.tile([P, NT, 1], F32, tag="bdec")
            gc = base.tile([P, NT, 1], F32, tag="gc")
            bc = base.tile([P, NT, 1], F32, tag="bc")
            mtc = base.tile([P, NT, 1], F32, tag="mtc")
            msc = base.tile([P, NT, 1], F32, tag="msc")
            onesb = base.tile([P, 1], BF16, tag="onesb")
            onesr = base.tile([1, P], F32R, tag="onesr")
            epsc = base.tile([P, 1], F32, tag="epsc")

            # ---- startup: spread initial DMAs across engine queues ----
            nc.sync.dma_start(out=x8[:, :, :], in_=dec8_d.ap())
            nc.gpsimd.dma_start(out=wk[:, :, :, :], in_=wk_d.ap())
            nc.scalar.dma_start(out=wv[:, :, :], in_=wv_d.ap())
            nc.scalar.dma_start(out=dqb[:, :, :], in_=dqb_d.ap())
            nc.scalar.dma_start(out=x8e[:, :, :], in_=enc8_d.ap())
            nc.scalar.dma_start(
                out=wdh[:, :, :, :],
                in_=wd_d.ap().rearrange("o p i c -> p o i c")[:, :, 4:NT, :])
            nc.gpsimd.dma_start(out=bqc[:, :, :], in_=bq_d.ap())
            nc.gpsimd.dma_start(out=bdec[:, :, :], in_=bde_d.ap())
            nc.gpsimd.dma_start(out=gc[:, :, :], in_=lng_d.ap())
            nc.gpsimd.dma_start(out=bc[:, :, :], in_=lnb_d.ap())
            nc.gpsimd.dma_start(out=mtc[:, :, :], in_=mt_d.ap())
            nc.gpsimd.dma_start(out=msc[:, :, :], in_=ms_d.ap())
            nc.gpsimd.dma_start(out=onesr[:, :], in_=onesrr_d.ap().bitcast(F32R))
            nc.gpsimd.memset(onesb[:, :], 1.0)
            nc.vector.memset(epsc[:, :], EPS)
            # softmax-denominator ones rows of the PV stationary blocks
            for st in range(NT):
                nc.gpsimd.memset(vi[:, st, :, D:P], 1.0)

            prp = tc.alloc_tile_pool(name="prp", bufs=4)
            rcp = tc.alloc_tile_pool(name="rcp", bufs=3)

            def k_unit(src, ot, ps, uid):
                """One o-tile of a k projection: fp8 DoubleRow matmuls,
                descaled eviction into kT (no k bias needed)."""
                for tch in range(2):
                    tsl = slice(tch * R, (tch + 1) * R)
                    pk = ps.tile([P, R], F32, tag="pk", name=f"pk{uid}_{ot}_{tch}")
                    for i in range(4):
                        nc.tensor.matmul(
                            pk[:, :], wk[:, ot, 2 * i:2 * i + 2, :],
                            src[:, 2 * i:2 * i + 2, tsl],
                            start=(i == 0), stop=(i == 3), perf_mode=DR)
                    nc.vector.tensor_scalar_mul(kT[:, ot, tsl], pk[:, :], INV)

            def v_unit(src, kb, ps, uid):
                """One key-block of a v projection, computed directly in
                [keys, vdim] layout (stationary = fp8 activation chunk)."""
                ksl = slice(kb * P, (kb + 1) * P)
                for hf in range(2):
                    vsl = slice(hf * R, (hf + 1) * R)
                    pv = ps.tile([P, R], F32, tag="pk", name=f"pv{uid}_{kb}_{hf}")
                    for i in range(4):
                        nc.tensor.matmul(
                            pv[:, :], src[:, 2 * i:2 * i + 2, ksl],
                            wv[:, 2 * i:2 * i + 2, vsl],
                            start=(i == 0), stop=(i == 3), perf_mode=DR)
                    nc.scalar.activation(
                        vi[:, kb, hf * 8:(hf + 1) * 8, 0:D],
                        pv[:, :].rearrange("p (h c) -> p h c", c=D),
                        AF.Identity, scale=INV)

            def q_unit(ot, ps, wp):
                """One o-tile of the q projection (bf16, biased)."""
                wqc = wp.tile([P, NT, P], BF16, tag="wqc", name=f"wqc_{ot}")
                nc.gpsimd.dma_start(out=wqc[:, :, :], in_=wq_d.ap()[ot])
                pq = ps.tile([P, R], F32, tag="pk", name=f"pq_{ot}")
                for it in range(NT):
                    nc.tensor.matmul(
                        pq[:, :], wqc[:, it, :], dqb[:, it, :],
                        start=(it == 0), stop=(it == NT - 1))
                nc.scalar.activation(
                    q1b[ot][:, :], pq[:, :], AF.Identity, bias=bqc[:, ot, :])

            def attention(qsrc, mcol, fillers, uid, min_pair=0):
                """scoresT -> batched exp -> fused PV+denominator -> ctxn.
                fillers: callbacks emitting independent PE work, drained
                across pair iterations (only once j >= min_pair)."""
                fill_i = 0
                with tc.tile_pool(name="psc", bufs=2, space="PSUM") as psc, \
                     tc.tile_pool(name="pcx", bufs=1, space="PSUM") as pcx:
                    for j in range(NH // 2):
                        c0 = pcx.tile([P, R], F32, tag="c0", name=f"c0{uid}_{j}")
                        c1 = pcx.tile([P, R], F32, tag="c1", name=f"c1{uid}_{j}")
                        probs = [None] * NT
                        for st in range(NT + 1):
                            # scores + exp for step st; PV for step st-1 (SW
                            # pipeline so the PE never waits on the current exp)
                            if st < NT:
                                ssl = slice(st * P, (st + 1) * P)
                                s01 = psc.tile([P, 2, R], F32, tag="s01", name=f"s{uid}_{j}_{st}")
                                nc.tensor.matmul(
                                    s01[:, 0, :], kT[0:D, j, ssl], qsrc[j][0:D, :])
                                nc.tensor.matmul(
                                    s01[:, 1, :], kT[D:P, j, ssl], qsrc[j][D:P, :])
                                p01 = prp.tile([P, 2, R], BF16, tag="p01", name=f"p{uid}_{j}_{st}")
                                nc.scalar.activation(
                                    p01[:, :, :], s01[:, :, :], AF.Exp,
                                    bias=mcol[:, st, :], scale=0.125)
                                probs[st] = p01
                            if st > 0:
                                pp01 = probs[st - 1]
                                nc.tensor.matmul(
                                    c0[:, :], vi[:, st - 1, 2 * j, :], pp01[:, 0, :],
                                    start=(st == 1), stop=(st == NT))
                                nc.tensor.matmul(
                                    c1[:, :], vi[:, st - 1, 2 * j + 1, :], pp01[:, 1, :],
                                    start=(st == 1), stop=(st == NT))

                        cc0 = rcp.tile([P, R], F32, tag="cc", bufs=2, name=f"cc0{uid}_{j}")
                        cc1 = rcp.tile([P, R], F32, tag="cc", bufs=2, name=f"cc1{uid}_{j}")
                        # DVE copy frees the PSUM bank; keeps Scalar on exp
                        nc.vector.tensor_copy(cc0[:, :], c0[:, :])
                        nc.vector.tensor_copy(cc1[:, :], c1[:, :])
                        d0 = rcp.tile([D, R], F32, tag="rr", bufs=6, name=f"d0{uid}_{j}")
                        d1 = rcp.tile([D, R], F32, tag="rr", bufs=6, name=f"d1{uid}_{j}")
                        r0 = rcp.tile([D, R], F32, tag="rr", bufs=6, name=f"r0{uid}_{j}")
                        r1 = rcp.tile([D, R], F32, tag="rr", bufs=6, name=f"r1{uid}_{j}")
                        nc.vector.tensor_copy(d0[:, :], cc0[D:P, :])
                        nc.vector.reciprocal_approx_fast(r0[:, :], d0[:, :])
                        nc.vector.tensor_mul(ctxn[0:D, j, :], cc0[0:D, :], r0[:, :])
                        nc.vector.tensor_copy(d1[:, :], cc1[D:P, :])
                        nc.vector.reciprocal_approx_fast(r1[:, :], d1[:, :])
                        nc.vector.tensor_mul(ctxn[D:P, j, :], cc1[0:D, :], r1[:, :])
                        navail = NH // 2 - min_pair
                        while (fillers and j >= min_pair and
                               fill_i < (j - min_pair + 1) * len(fillers) // navail):
                            fillers[fill_i]()
                            fill_i += 1
                    while fill_i < len(fillers):
                        fillers[fill_i]()
                        fill_i += 1

            def proj_ln(resid_b, dst, fillers, uid, partial=None, hold=0,
                        store=False):
                """Out-projection + residual into resT with LN stats fused
                per o-tile; then row stats, broadcast, per-o-tile apply -> dst.
                If partial is given, it holds ht 0..3 of the accumulation and
                only ht 4..7 run here. Held fillers drain BEFORE the broadcast
                matmuls so the PE queue isn't blocked on the stats chain."""
                fill_i = 0
                with tc.tile_pool(name="wdp", bufs=2) as wdp, \
                     tc.tile_pool(name="sqp", bufs=2) as sqp, \
                     tc.tile_pool(name="lnp", bufs=1) as lnp, \
                     tc.tile_pool(name="outp", bufs=2) as outp, \
                     tc.tile_pool(name="ps3", bufs=2, space="PSUM") as ps, \
                     tc.tile_pool(name="ps4", bufs=1, space="PSUM") as ps4:
                    pmu = ps4.tile([1, R], F32, tag="pmu", name=f"pmu{uid}")
                    psq = ps4.tile([1, R], F32, tag="psq", name=f"psq{uid}")
                    for ot in range(NT):
                        pp = ps.tile([P, R], F32, tag="pp", name=f"pp{uid}_{ot}")
                        if partial is None:
                            wdc = wdp.tile([P, NT, P], F8, tag="wd", name=f"wd{uid}_{ot}")
                            nc.sync.dma_start(out=wdc[:, :, :], in_=wd_d.ap()[ot])
                            for i in range(4):
                                nc.tensor.matmul(
                                    pp[:, :], wdc[:, 2 * i:2 * i + 2, :],
                                    ctxn[:, 2 * i:2 * i + 2, :],
                                    start=(i == 0), stop=(i == 3), perf_mode=DR)
                            tmp = sqp.tile([P, R], F32, tag="ts", name=f"tmp{uid}_{ot}", bufs=2)
                            nc.vector.tensor_scalar(
                                tmp[:, :], pp[:, :], INV, bdec[:, ot, :],
                                op0=ALU.mult, op1=ALU.add)
                            nc.vector.tensor_add(
                                resT[:, ot, :], tmp[:, :], resid_b[ot][:, :])
                        else:
                            for i in range(2):
                                nc.tensor.matmul(
                                    pp[:, :], wdh[:, ot, 2 * i:2 * i + 2, :],
                                    ctxn[:, 2 * i + 4:2 * i + 6, :],
                                    start=(i == 0), stop=(i == 1), perf_mode=DR)
                            tsum = sqp.tile([P, R], F32, tag="ts", name=f"tsum{uid}_{ot}", bufs=2)
                            nc.vector.scalar_tensor_tensor(
                                tsum[:, :], pp[:, :], INV,
                                partial[:, ot, :], op0=ALU.mult, op1=ALU.add)
                            nc.vector.tensor_add(
                                resT[:, ot, :], tsum[:, :], resid_b[ot][:, :])
                        sq = sqp.tile([P, R], BF16, tag="sq", name=f"sq{uid}_{ot}")
                        nc.scalar.square(sq[:, :], resT[:, ot, :])
                        nc.tensor.matmul(
                            pmu[:, :], onesb[:, :], resT[:, ot, :],
                            start=(ot == 0), stop=(ot == NT - 1))
                        nc.tensor.matmul(
                            psq[:, :], onesb[:, :], sq[:, :],
                            start=(ot == 0), stop=(ot == NT - 1))
                        early = len(fillers) - hold
                        while fillers and fill_i < (ot + 1) * early // NT:
                            fillers[fill_i]()
                            fill_i += 1
                    # held fillers drain now: independent PE work queued ahead
                    # of the broadcast matmuls that wait on the stats chain
                    while fill_i < len(fillers):
                        fillers[fill_i]()
                        fill_i += 1
                    mu_r = lnp.tile([1, R], F32R, tag="lnrow", bufs=2, name=f"mu{uid}")
                    nc.scalar.mul(mu_r[:, :], pmu[:, :], 1.0 / H)
                    sq_r = lnp.tile([1, R], F32R, tag="lnrow", bufs=2, name=f"sqr{uid}")
                    nc.scalar.mul(sq_r[:, :], psq[:, :], 1.0 / H)
                    muB = ps4.tile([P, R], F32, tag="pmu", name=f"muBp{uid}")
                    nc.tensor.matmul(muB[:, :], onesr[:, :], mu_r[:, :])
                    sqBp = ps4.tile([P, R], F32, tag="psq", name=f"sqBp{uid}")
                    nc.tensor.matmul(sqBp[:, :], onesr[:, :], sq_r[:, :])
                    # free the PSUM banks fast: msB/muBb read muB, varB reads
                    # sqBp, then the whole apply runs from SBUF in bf16 (2x DVE)
                    msB = sqp.tile([P, R], F32, tag="lnB", name=f"msB{uid}", bufs=2)
                    nc.scalar.square(msB[:, :], muB[:, :])
                    muBb = sqp.tile([P, R], BF16, tag="muBb", name=f"muBb{uid}", bufs=1)
                    nc.vector.tensor_copy(muBb[:, :], muB[:, :])
                    varB = sqp.tile([P, R], F32, tag="lnB", name=f"varB{uid}", bufs=2)
                    nc.vector.tensor_sub(varB[:, :], sqBp[:, :], msB[:, :])
                    sdB = sqp.tile([P, R], F32, tag="lnB", name=f"sdB{uid}", bufs=2)
                    nc.scalar.activation(sdB[:, :], varB[:, :], AF.Sqrt, bias=epsc[:, :])
                    rsB = sqp.tile([P, R], F32, tag="rsB", name=f"rsB{uid}", bufs=1)
                    nc.vector.reciprocal_approx_fast(rsB[:, :], sdB[:, :])
                    rsBb = sqp.tile([P, R], BF16, tag="rsBb", name=f"rsBb{uid}", bufs=1)
                    nc.vector.tensor_copy(rsBb[:, :], rsB[:, :])
                    for ot in range(NT):
                        t1 = sqp.tile([P, R], BF16, tag="tt", name=f"t1{uid}_{ot}", bufs=4)
                        nc.vector.tensor_sub(t1[:, :], resT[:, ot, :], muBb[:, :])
                        t2 = sqp.tile([P, R], BF16, tag="tt", name=f"t2{uid}_{ot}", bufs=4)
                        nc.vector.tensor_mul(t2[:, :], t1[:, :], rsBb[:, :])
                        if store:
                            oT = outp.tile([P, R], F32, tag="oT", name=f"oT{uid}_{ot}")
                            nc.scalar.activation(
                                oT[:, :], t2[:, :], AF.Identity,
                                bias=bc[:, ot, :], scale=gc[:, ot, :])
                            eng = (nc.sync, nc.scalar, nc.gpsimd)[ot % 3]
                            eng.dma_start(out=out_d.ap()[:, ot, :], in_=oT[:, :])
                        else:
                            nc.gpsimd.tensor_scalar(
                                dst[ot][:, :], t2[:, :], gc[:, ot, :], bc[:, ot, :],
                                op0=ALU.mult, op1=ALU.add)

            # ================== phase 1: decoder projections ==================
            # ===== + phase 2: self-attn (k/q tail and encoder-k as fills) =====
            with tc.tile_pool(name="wqp", bufs=3) as wqp, \
                 tc.tile_pool(name="psA", bufs=2, space="PSUM") as psA:
                k_unit(x8, 0, psA, "a")
                k_unit(x8, 1, psA, "a")
                for kb in range(NT):
                    v_unit(x8, kb, psA, "a")
                q_unit(0, psA, wqp)
                q_unit(1, psA, wqp)

                fills = []
                for ot in range(2, NT):
                    fills.append(lambda ot=ot: k_unit(x8, ot, psA, "a"))
                    fills.append(lambda ot=ot: q_unit(ot, psA, wqp))
                for ot in range(NT):
                    fills.append(lambda ot=ot: k_unit(x8e, ot, psA, "b"))
                attention(q1b, mtc, fills, "A")

            # ========= phase 3: out-proj + LN1 (+ encoder-v interleaved) =====
            with tc.tile_pool(name="psV", bufs=2, space="PSUM") as psV:
                evs = [lambda kb=kb: v_unit(x8e, kb, psV, "b") for kb in range(NT)]
                proj_ln(q1b, slfb, evs, "A", hold=4)

            # ==================== phase 4: cross-attention ====================
            with tc.tile_pool(name="wdpB", bufs=2) as wdpB, \
                 tc.tile_pool(name="psB", bufs=2, space="PSUM") as psB:
                def mk_pA(ot):
                    def f():
                        wdc = wdpB.tile([P, 4, P], F8, tag="wdA", name=f"wdA{ot}")
                        nc.sync.dma_start(out=wdc[:, :, :], in_=wd_d.ap()[ot][:, 0:4, :])
                        pp = psB.tile([P, R], F32, tag="ppA", name=f"ppA{ot}")
                        for i in range(2):
                            nc.tensor.matmul(
                                pp[:, :], wdc[:, 2 * i:2 * i + 2, :],
                                ctxn[:, 2 * i:2 * i + 2, :],
                                start=(i == 0), stop=(i == 1), perf_mode=DR)
                        nc.vector.tensor_scalar(
                            partialA[:, ot, :], pp[:, :], INV, bdec[:, ot, :],
                            op0=ALU.mult, op1=ALU.add)
                    return f

                attention(slfb, msc, [mk_pA(ot) for ot in range(NT)], "B", min_pair=4)

            # ============ phase 5: out-proj + LN2 + store ====================
            proj_ln(slfb, None, [], "B", partial=partialA, store=True)
            rcp.release()
            prp.release()

    nc.compile()
    return nc


_NC = None

_F8NP = ml_dtypes.float8_e4m3
_BFNP = ml_dtypes.bfloat16
_ONESR = np.ones((1, P), np.float32)


def make_in_maps(encoder_states, decoder_inputs, src_attention_mask,
                 tgt_attention_mask, Wq, bq, Wk, bk, Wv, bv, Wd, bd, ln_g, ln_b):
    f = np.float32

    def wtile(w, dt, scale=1.0):  # [o,i] -> W.T chunks [ot, p_i, it, p_o]
        a = (np.asarray(w, f).T * scale).reshape(NT, P, NT, P)
        return np.ascontiguousarray(a.transpose(2, 1, 0, 3)).astype(dt)

    def atile(x, dt):  # [t,i] -> x.T tiled [p, it, t]
        return np.ascontiguousarray(
            np.asarray(x, f).T.reshape(NT, P, -1).transpose(1, 0, 2)).astype(dt)

    col = lambda x: np.ascontiguousarray(
        np.asarray(x, f).reshape(NT, P).T.reshape(P, NT, 1))

    wq_t = wtile(Wq, _BFNP)
    wd_t = wtile(Wd, _F8NP, WS)
    # wk resident layout [p_i, ot, it, p_o]
    wk_t = np.ascontiguousarray(
        wtile(Wk, np.float32, WS).transpose(1, 0, 2, 3)).astype(_F8NP)
    # wv v-direct layout [p_i, it, o]
    wv_t = np.ascontiguousarray(
        (np.asarray(Wv, f).T * WS).reshape(NT, P, H).transpose(1, 0, 2)
    ).astype(_F8NP)
    bde = np.asarray(bd, f) + np.asarray(bv, f) @ np.asarray(Wd, f).T
    bq_, bde_ = col(bq), col(bde)
    g_, b_ = col(ln_g), col(ln_b)

    dec8_b = [atile(decoder_inputs[b], _F8NP) for b in range(B)]
    enc8_b = [atile(encoder_states[b], _F8NP) for b in range(B)]
    mt_b = [col(tgt_attention_mask[b, 0, 0, :]) for b in range(B)]
    ms_b = [col(src_attention_mask[b, 0, 0, :]) for b in range(B)]

    in_maps = []
    for c in range(8):
        b, half = c // 2, c % 2
        in_maps.append({
            "dec8": dec8_b[b],
            "enc8": enc8_b[b],
            "dqb": atile(decoder_inputs[b][half * R:(half + 1) * R], _BFNP),
            "wq": wq_t, "wk": wk_t, "wv": wv_t, "wd": wd_t,
            "bq": bq_, "bde": bde_,
            "lng": g_, "lnb": b_,
            "mt": mt_b[b], "ms": ms_b[b],
            "onesr": _ONESR,
        })
    return in_maps


def kernel(**inputs):
    global _NC
    if _NC is None:
        _NC = build_kernel()
    nc = _NC
    in_maps = make_in_maps(**inputs)
    res = run_bass_kernel_spmd(nc, in_maps, core_ids=list(range(8)))
    out = np.empty((B, T, H), np.float32)
    for c in range(8):
        b, half = c // 2, c % 2
        buf = res.results[c]["out"]  # [p, ot, t]
        out[b, half * R:(half + 1) * R, :] = (
            buf.transpose(2, 1, 0).reshape(R, H))
    return out


# revision 28
# speedup vs baseline: 1.1530x; 1.1530x over previous
"""Trainium2 Bass kernel for an ALBERT-style seq2seq block (self-attn + cross-attn).

Sharding: 8 cores = (batch b in 0..3) x (decoder-row half in 0..1); zero
inter-core communication. Each core computes its 512 decoder rows.

v2 design vs baseline:
- All activations/weights in bf16; k/v projections in fp8(e4m3) with
  DoubleRow perf-mode matmuls (2 k-subtiles per pass, 0.5 cyc/col).
- v computed directly in [keys, vdim] layout (stationary = activation
  chunk), eliminating all PE transposes.
- k bias dropped (softmax shift-invariant), v bias folded into the
  out-projection bias on the host (bd_eff = bd + bv @ Wd.T).
- PSUM evictions and softmax-denominator copies moved off the Scalar
  engine (exp is the Scalar bottleneck) to GpSimd/Vector.
- Leaner LN tail (Rsqrt fused); held fillers drain BEFORE the broadcast
  matmuls so the PE queue is never head-of-line blocked on LN stats.
"""

import sys

sys.path.insert(0, "/opt/trn_rl_repo")

import numpy as np
import ml_dtypes

import concourse.bacc as bacc
import concourse.mybir as mybir
from concourse.bass_utils import run_bass_kernel_spmd
from concourse.tile import TileContext

F32 = mybir.dt.float32
F32R = mybir.dt.float32r
BF16 = mybir.dt.bfloat16
F8 = mybir.dt.float8e4
AF = mybir.ActivationFunctionType
ALU = mybir.AluOpType
DR = mybir.MatmulPerfMode.DoubleRow

P = 128          # partitions
H = 1024         # hidden
NT = H // P      # 8 tiles over hidden
NH = 16          # heads
D = 64           # head dim
T = 1024         # sequence length (encoder and decoder)
R = 512          # decoder rows per core
B = 4
EPS = 1e-12
WS = 32.0        # fp8 weight pre-scale (host); descaled at PSUM eviction
INV = 1.0 / WS


def build_kernel():
    nc = bacc.Bacc("TRN2", num_devices=8)

    dec8_d = nc.declare_dram_parameter("dec8", [P, NT, T], F8, isOutput=False)
    enc8_d = nc.declare_dram_parameter("enc8", [P, NT, T], F8, isOutput=False)
    dqb_d = nc.declare_dram_parameter("dqb", [P, NT, R], BF16, isOutput=False)
    wq_d = nc.declare_dram_parameter("wq", [NT, P, NT, P], BF16, isOutput=False)
    wk_d = nc.declare_dram_parameter("wk", [P, NT, NT, P], F8, isOutput=False)
    wv_d = nc.declare_dram_parameter("wv", [P, NT, H], F8, isOutput=False)
    wd_d = nc.declare_dram_parameter("wd", [NT, P, NT, P], F8, isOutput=False)
    bq_d = nc.declare_dram_parameter("bq", [P, NT, 1], F32, isOutput=False)
    bde_d = nc.declare_dram_parameter("bde", [P, NT, 1], F32, isOutput=False)
    lng_d = nc.declare_dram_parameter("lng", [P, NT, 1], F32, isOutput=False)
    lnb_d = nc.declare_dram_parameter("lnb", [P, NT, 1], F32, isOutput=False)
    mt_d = nc.declare_dram_parameter("mt", [P, NT, 1], F32, isOutput=False)
    ms_d = nc.declare_dram_parameter("ms", [P, NT, 1], F32, isOutput=False)
    onesrr_d = nc.declare_dram_parameter("onesr", [1, P], F32, isOutput=False)
    out_d = nc.declare_dram_parameter("out", [P, NT, R], F32, isOutput=True)

    with TileContext(nc) as tc:
        with tc.tile_pool(name="base", bufs=1) as base:
            x8 = base.tile([P, NT, T], F8, tag="x8")
            x8e = base.tile([P, NT, T], F8, tag="x8e")
            dqb = base.tile([P, NT, R], BF16, tag="dqb")
            wk = base.tile([P, NT, NT, P], F8, tag="wk")
            wv = base.tile([P, NT, H], F8, tag="wv")
            kT = base.tile([P, NT, T], BF16, tag="kT")
            vi = base.tile([P, NT, NH, P], BF16, tag="vi")
            q1b = [base.tile([P, R], BF16, tag=f"q1b{o}", name=f"q1b{o}") for o in range(NT)]
            slfb = [base.tile([P, R], BF16, tag=f"slfb{o}", name=f"slfb{o}") for o in range(NT)]
            ctxn = base.tile([P, NT, R], F8, tag="ctxn")
            resT = base.tile([P, NT, R], BF16, tag="resT")
            partialA = base.tile([P, NT, R], F32, tag="partialA")
            wdh = base.tile([P, NT, 4, P], F8, tag="wdh")
            bqc = base.tile([P, NT, 1], F32, tag="bqc")
            bdec = base.tile([P, NT, 1], F32, tag="bdec")
            gc = base.tile([P, NT, 1], F32, tag="gc")
            bc = base.tile([P, NT, 1], F32, tag="bc")
            mtc = base.tile([P, NT, 1], F32, tag="mtc")
            msc = base.tile([P, NT, 1], F32, tag="msc")
            onesb = base.tile([P, 1], BF16, tag="onesb")
            onesr = base.tile([1, P], F32R, tag="onesr")
            epsc = base.tile([P, 1], F32, tag="epsc")

            # ---- startup: spread initial DMAs across engine queues ----
            nc.sync.dma_start(out=x8[:, 0:4, :], in_=dec8_d.ap()[:, 0:4, :])
            nc.scalar.dma_start(out=x8[:, 4:NT, :], in_=dec8_d.ap()[:, 4:NT, :])
            nc.gpsimd.dma_start(out=wk[:, :, :, :], in_=wk_d.ap())
            nc.sync.dma_start(out=wv[:, :, :], in_=wv_d.ap())
            nc.scalar.dma_start(out=dqb[:, :, :], in_=dqb_d.ap())
            nc.scalar.dma_start(out=x8e[:, :, :], in_=enc8_d.ap())
            nc.scalar.dma_start(
                out=wdh[:, :, :, :],
                in_=wd_d.ap().rearrange("o p i c -> p o i c")[:, :, 4:NT, :])
            nc.gpsimd.dma_start(out=bqc[:, :, :], in_=bq_d.ap())
            nc.gpsimd.dma_start(out=bdec[:, :, :], in_=bde_d.ap())
            nc.gpsimd.dma_start(out=gc[:, :, :], in_=lng_d.ap())
            nc.gpsimd.dma_start(out=bc[:, :, :], in_=lnb_d.ap())
            nc.gpsimd.dma_start(out=mtc[:, :, :], in_=mt_d.ap())
            nc.gpsimd.dma_start(out=msc[:, :, :], in_=ms_d.ap())
            nc.gpsimd.dma_start(out=onesr[:, :], in_=onesrr_d.ap().bitcast(F32R))
            nc.gpsimd.memset(onesb[:, :], 1.0)
            nc.vector.memset(epsc[:, :], EPS)
            # softmax-denominator ones rows of the PV stationary blocks
            for st in range(NT):
                nc.gpsimd.memset(vi[:, st, :, D:P], 1.0)

            prp = tc.alloc_tile_pool(name="prp", bufs=4)
            rcp = tc.alloc_tile_pool(name="rcp", bufs=3)

            def k_unit(src, ot, ps, uid):
                """One o-tile of a k projection: fp8 DoubleRow matmuls,
                descaled eviction into kT (no k bias needed)."""
                for tch in range(2):
                    tsl = slice(tch * R, (tch + 1) * R)
                    pk = ps.tile([P, R], F32, tag="pk", name=f"pk{uid}_{ot}_{tch}")
                    for i in range(4):
                        nc.tensor.matmul(
                            pk[:, :], wk[:, ot, 2 * i:2 * i + 2, :],
                            src[:, 2 * i:2 * i + 2, tsl],
                            start=(i == 0), stop=(i == 3), perf_mode=DR)
                    nc.vector.tensor_scalar_mul(kT[:, ot, tsl], pk[:, :], INV)

            def v_unit(src, kb, ps, uid):
                """One key-block of a v projection, computed directly in
                [keys, vdim] layout (stationary = fp8 activation chunk)."""
                ksl = slice(kb * P, (kb + 1) * P)
                for hf in range(2):
                    vsl = slice(hf * R, (hf + 1) * R)
                    pv = ps.tile([P, R], F32, tag="pk", name=f"pv{uid}_{kb}_{hf}")
                    for i in range(4):
                        nc.tensor.matmul(
                            pv[:, :], src[:, 2 * i:2 * i + 2, ksl],
                            wv[:, 2 * i:2 * i + 2, vsl],
                            start=(i == 0), stop=(i == 3), perf_mode=DR)
                    nc.scalar.activation(
                        vi[:, kb, hf * 8:(hf + 1) * 8, 0:D],
                        pv[:, :].rearrange("p (h c) -> p h c", c=D),
                        AF.Identity, scale=INV)

            def q_unit(ot, ps, wp):
                """One o-tile of the q projection (bf16, biased)."""
                wqc = wp.tile([P, NT, P], BF16, tag="wqc", name=f"wqc_{ot}")
                nc.gpsimd.dma_start(out=wqc[:, :, :], in_=wq_d.ap()[ot])
                pq = ps.tile([P, R], F32, tag="pk", name=f"pq_{ot}")
                for it in range(NT):
                    nc.tensor.matmul(
                        pq[:, :], wqc[:, it, :], dqb[:, it, :],
                        start=(it == 0), stop=(it == NT - 1))
                nc.scalar.activation(
                    q1b[ot][:, :], pq[:, :], AF.Identity, bias=bqc[:, ot, :])

            def attention(qsrc, mcol, fillers, uid, min_pair=0):
                """scoresT -> batched exp -> fused PV+denominator -> ctxn.
                fillers: callbacks emitting independent PE work, drained
                across pair iterations (only once j >= min_pair)."""
                fill_i = 0
                with tc.tile_pool(name="psc", bufs=2, space="PSUM") as psc, \
                     tc.tile_pool(name="pcx", bufs=1, space="PSUM") as pcx:
                    for j in range(NH // 2):
                        c0 = pcx.tile([P, R], F32, tag="c0", name=f"c0{uid}_{j}")
                        c1 = pcx.tile([P, R], F32, tag="c1", name=f"c1{uid}_{j}")
                        probs = [None] * NT
                        for st in range(NT + 1):
                            # scores + exp for step st; PV for step st-1 (SW
                            # pipeline so the PE never waits on the current exp)
                            if st < NT:
                                ssl = slice(st * P, (st + 1) * P)
                                s01 = psc.tile([P, 2, R], F32, tag="s01", name=f"s{uid}_{j}_{st}")
                                nc.tensor.matmul(
                                    s01[:, 0, :], kT[0:D, j, ssl], qsrc[j][0:D, :])
                                nc.tensor.matmul(
                                    s01[:, 1, :], kT[D:P, j, ssl], qsrc[j][D:P, :])
                                p01 = prp.tile([P, 2, R], BF16, tag="p01", name=f"p{uid}_{j}_{st}")
                                nc.scalar.activation(
                                    p01[:, :, :], s01[:, :, :], AF.Exp,
                                    bias=mcol[:, st, :], scale=0.125)
                                probs[st] = p01
                            if st > 0:
                                pp01 = probs[st - 1]
                                nc.tensor.matmul(
                                    c0[:, :], vi[:, st - 1, 2 * j, :], pp01[:, 0, :],
                                    start=(st == 1), stop=(st == NT))
                                nc.tensor.matmul(
                                    c1[:, :], vi[:, st - 1, 2 * j + 1, :], pp01[:, 1, :],
                                    start=(st == 1), stop=(st == NT))

                        cc0 = rcp.tile([P, R], F32, tag="cc", bufs=2, name=f"cc0{uid}_{j}")
                        cc1 = rcp.tile([P, R], F32, tag="cc", bufs=2, name=f"cc1{uid}_{j}")
                        # DVE copy frees the PSUM bank; keeps Scalar on exp
                        nc.vector.tensor_copy(cc0[:, :], c0[:, :])
                        nc.vector.tensor_copy(cc1[:, :], c1[:, :])
                        d0 = rcp.tile([D, R], F32, tag="rr", bufs=6, name=f"d0{uid}_{j}")
                        d1 = rcp.tile([D, R], F32, tag="rr", bufs=6, name=f"d1{uid}_{j}")
                        r0 = rcp.tile([D, R], F32, tag="rr", bufs=6, name=f"r0{uid}_{j}")
                        r1 = rcp.tile([D, R], F32, tag="rr", bufs=6, name=f"r1{uid}_{j}")
                        nc.vector.tensor_copy(d0[:, :], cc0[D:P, :])
                        nc.vector.reciprocal_approx_fast(r0[:, :], d0[:, :])
                        nc.vector.tensor_mul(ctxn[0:D, j, :], cc0[0:D, :], r0[:, :])
                        nc.vector.tensor_copy(d1[:, :], cc1[D:P, :])
                        nc.vector.reciprocal_approx_fast(r1[:, :], d1[:, :])
                        nc.vector.tensor_mul(ctxn[D:P, j, :], cc1[0:D, :], r1[:, :])
                        navail = NH // 2 - min_pair
                        while (fillers and j >= min_pair and
                               fill_i < (j - min_pair + 1) * len(fillers) // navail):
                            fillers[fill_i]()
                            fill_i += 1
                    while fill_i < len(fillers):
                        fillers[fill_i]()
                        fill_i += 1

            def proj_ln(resid_b, dst, fillers, uid, partial=None, hold=0,
                        store=False):
                """Out-projection + residual into resT with LN stats fused
                per o-tile; then row stats, broadcast, per-o-tile apply -> dst.
                If partial is given, it holds ht 0..3 of the accumulation and
                only ht 4..7 run here. Held fillers drain BEFORE the broadcast
                matmuls so the PE queue isn't blocked on the stats chain."""
                fill_i = 0
                with tc.tile_pool(name="wdp", bufs=2) as wdp, \
                     tc.tile_pool(name="sqp", bufs=2) as sqp, \
                     tc.tile_pool(name="lnp", bufs=1) as lnp, \
                     tc.tile_pool(name="outp", bufs=2) as outp, \
                     tc.tile_pool(name="ps3", bufs=2, space="PSUM") as ps, \
                     tc.tile_pool(name="ps4", bufs=1, space="PSUM") as ps4:
                    pmu = ps4.tile([1, R], F32, tag="pmu", name=f"pmu{uid}")
                    psq = ps4.tile([1, R], F32, tag="psq", name=f"psq{uid}")
                    for ot in range(NT):
                        pp = ps.tile([P, R], F32, tag="pp", name=f"pp{uid}_{ot}")
                        if partial is None:
                            wdc = wdp.tile([P, NT, P], F8, tag="wd", name=f"wd{uid}_{ot}")
                            nc.sync.dma_start(out=wdc[:, :, :], in_=wd_d.ap()[ot])
                            for i in range(4):
                                nc.tensor.matmul(
                                    pp[:, :], wdc[:, 2 * i:2 * i + 2, :],
                                    ctxn[:, 2 * i:2 * i + 2, :],
                                    start=(i == 0), stop=(i == 3), perf_mode=DR)
                            tmp = sqp.tile([P, R], F32, tag="ts", name=f"tmp{uid}_{ot}", bufs=2)
                            nc.vector.tensor_scalar(
                                tmp[:, :], pp[:, :], INV, bdec[:, ot, :],
                                op0=ALU.mult, op1=ALU.add)
                            nc.vector.tensor_add(
                                resT[:, ot, :], tmp[:, :], resid_b[ot][:, :])
                        else:
                            for i in range(2):
                                nc.tensor.matmul(
                                    pp[:, :], wdh[:, ot, 2 * i:2 * i + 2, :],
                                    ctxn[:, 2 * i + 4:2 * i + 6, :],
                                    start=(i == 0), stop=(i == 1), perf_mode=DR)
                            tsum = sqp.tile([P, R], F32, tag="ts", name=f"tsum{uid}_{ot}", bufs=2)
                            nc.vector.scalar_tensor_tensor(
                                tsum[:, :], pp[:, :], INV,
                                partial[:, ot, :], op0=ALU.mult, op1=ALU.add)
                            nc.vector.tensor_add(
                                resT[:, ot, :], tsum[:, :], resid_b[ot][:, :])
                        sq = sqp.tile([P, R], BF16, tag="sq", name=f"sq{uid}_{ot}")
                        nc.scalar.square(sq[:, :], resT[:, ot, :])
                        nc.tensor.matmul(
                            pmu[:, :], onesb[:, :], resT[:, ot, :],
                            start=(ot == 0), stop=(ot == NT - 1))
                        nc.tensor.matmul(
                            psq[:, :], onesb[:, :], sq[:, :],
                            start=(ot == 0), stop=(ot == NT - 1))
                        early = len(fillers) - hold
                        while fillers and fill_i < (ot + 1) * early // NT:
                            fillers[fill_i]()
                            fill_i += 1
                    # held fillers drain now: independent PE work queued ahead
                    # of the broadcast matmuls that wait on the stats chain
                    while fill_i < len(fillers):
                        fillers[fill_i]()
                        fill_i += 1
                    mu_r = lnp.tile([1, R], F32R, tag="lnrow", bufs=2, name=f"mu{uid}")
                    nc.scalar.mul(mu_r[:, :], pmu[:, :], 1.0 / H)
                    sq_r = lnp.tile([1, R], F32R, tag="lnrow", bufs=2, name=f"sqr{uid}")
                    nc.scalar.mul(sq_r[:, :], psq[:, :], 1.0 / H)
                    muB = ps4.tile([P, R], F32, tag="pmu", name=f"muBp{uid}")
                    nc.tensor.matmul(muB[:, :], onesr[:, :], mu_r[:, :])
                    sqBp = ps4.tile([P, R], F32, tag="psq", name=f"sqBp{uid}")
                    nc.tensor.matmul(sqBp[:, :], onesr[:, :], sq_r[:, :])
                    # free the PSUM banks fast: msB/muBb read muB, varB reads
                    # sqBp, then the whole apply runs from SBUF in bf16 (2x DVE)
                    msB = sqp.tile([P, R], F32, tag="lnB", name=f"msB{uid}", bufs=2)
                    nc.scalar.square(msB[:, :], muB[:, :])
                    muBb = sqp.tile([P, R], BF16, tag="muBb", name=f"muBb{uid}", bufs=1)
                    nc.vector.tensor_copy(muBb[:, :], muB[:, :])
                    varB = sqp.tile([P, R], F32, tag="lnB", name=f"varB{uid}", bufs=2)
                    nc.vector.tensor_sub(varB[:, :], sqBp[:, :], msB[:, :])
                    sdB = sqp.tile([P, R], F32, tag="lnB", name=f"sdB{uid}", bufs=2)
                    nc.scalar.activation(sdB[:, :], varB[:, :], AF.Sqrt, bias=epsc[:, :])
                    rsB = sqp.tile([P, R], F32, tag="rsB", name=f"rsB{uid}", bufs=1)
                    nc.vector.reciprocal_approx_fast(rsB[:, :], sdB[:, :])
                    rsBb = sqp.tile([P, R], BF16, tag="rsBb", name=f"rsBb{uid}", bufs=1)
                    nc.vector.tensor_copy(rsBb[:, :], rsB[:, :])
                    for ot in range(NT):
                        t1 = sqp.tile([P, R], BF16, tag="tt", name=f"t1{uid}_{ot}", bufs=4)
                        nc.vector.tensor_sub(t1[:, :], resT[:, ot, :], muBb[:, :])
                        t2 = sqp.tile([P, R], BF16, tag="tt", name=f"t2{uid}_{ot}", bufs=4)
                        nc.vector.tensor_mul(t2[:, :], t1[:, :], rsBb[:, :])
                        if store:
                            oT = outp.tile([P, R], F32, tag="oT", name=f"oT{uid}_{ot}")
                            nc.scalar.activation(
                                oT[:, :], t2[:, :], AF.Identity,
                                bias=bc[:, ot, :], scale=gc[:, ot, :])
                            nc.sync.dma_start(out=out_d.ap()[:, ot, :], in_=oT[:, :])
                        else:
                            nc.gpsimd.tensor_scalar(
                                dst[ot][:, :], t2[:, :], gc[:, ot, :], bc[:, ot, :],
                                op0=ALU.mult, op1=ALU.add)

            # ================== phase 1: decoder projections ==================
            # ===== + phase 2: self-attn (k/q tail and encoder-k as fills) =====
            with tc.tile_pool(name="wqp", bufs=3) as wqp, \
                 tc.tile_pool(name="psA", bufs=2, space="PSUM") as psA:
                k_unit(x8, 0, psA, "a")
                k_unit(x8, 1, psA, "a")
                for kb in range(NT):
                    v_unit(x8, kb, psA, "a")
                q_unit(0, psA, wqp)
                q_unit(1, psA, wqp)

                fills = []
                for ot in range(2, NT):
                    fills.append(lambda ot=ot: k_unit(x8, ot, psA, "a"))
                    fills.append(lambda ot=ot: q_unit(ot, psA, wqp))
                for ot in range(NT):
                    fills.append(lambda ot=ot: k_unit(x8e, ot, psA, "b"))
                attention(q1b, mtc, fills, "A")

            # ========= phase 3: out-proj + LN1 (+ encoder-v interleaved) =====
            with tc.tile_pool(name="psV", bufs=2, space="PSUM") as psV:
                evs = [lambda kb=kb: v_unit(x8e, kb, psV, "b") for kb in range(NT)]
                proj_ln(q1b, slfb, evs, "A", hold=4)

            # ==================== phase 4: cross-attention ====================
            with tc.tile_pool(name="wdpB", bufs=2) as wdpB, \
                 tc.tile_pool(name="psB", bufs=2, space="PSUM") as psB:
                def mk_pA(ot):
                    def f():
                        wdc = wdpB.tile([P, 4, P], F8, tag="wdA", name=f"wdA{ot}")
                        nc.sync.dma_start(out=wdc[:, :, :], in_=wd_d.ap()[ot][:, 0:4, :])
                        pp = psB.tile([P, R], F32, tag="ppA", name=f"ppA{ot}")
                        for i in range(2):
                            nc.tensor.matmul(
                                pp[:, :], wdc[:, 2 * i:2 * i + 2, :],
                                ctxn[:, 2 * i:2 * i + 2, :],
                                start=(i == 0), stop=(i == 1), perf_mode=DR)
                        nc.vector.tensor_scalar(
                            partialA[:, ot, :], pp[:, :], INV, bdec[:, ot, :],
                            op0=ALU.mult, op1=ALU.add)
                    return f

                attention(slfb, msc, [mk_pA(ot) for ot in range(NT)], "B", min_pair=4)

            # ============ phase 5: out-proj + LN2 + store ====================
            proj_ln(slfb, None, [], "B", partial=partialA, store=True)
            rcp.release()
            prp.release()

    nc.compile()
    return nc


_NC = None

_F8NP = ml_dtypes.float8_e4m3
_BFNP = ml_dtypes.bfloat16
_ONESR = np.ones((1, P), np.float32)


def make_in_maps(encoder_states, decoder_inputs, src_attention_mask,
                 tgt_attention_mask, Wq, bq, Wk, bk, Wv, bv, Wd, bd, ln_g, ln_b):
    f = np.float32

    def wtile(w, dt, scale=1.0):  # [o,i] -> W.T chunks [ot, p_i, it, p_o]
        a = (np.asarray(w, f).T * scale).reshape(NT, P, NT, P)
        return np.ascontiguousarray(a.transpose(2, 1, 0, 3)).astype(dt)

    def atile(x, dt):  # [t,i] -> x.T tiled [p, it, t]
        return np.ascontiguousarray(
            np.asarray(x, f).T.reshape(NT, P, -1).transpose(1, 0, 2)).astype(dt)

    col = lambda x: np.ascontiguousarray(
        np.asarray(x, f).reshape(NT, P).T.reshape(P, NT, 1))

    wq_t = wtile(Wq, _BFNP)
    wd_t = wtile(Wd, _F8NP, WS)
    # wk resident layout [p_i, ot, it, p_o]
    wk_t = np.ascontiguousarray(
        wtile(Wk, np.float32, WS).transpose(1, 0, 2, 3)).astype(_F8NP)
    # wv v-direct layout [p_i, it, o]
    wv_t = np.ascontiguousarray(
        (np.asarray(Wv, f).T * WS).reshape(NT, P, H).transpose(1, 0, 2)
    ).astype(_F8NP)
    bde = np.asarray(bd, f) + np.asarray(bv, f) @ np.asarray(Wd, f).T
    bq_, bde_ = col(bq), col(bde)
    g_, b_ = col(ln_g), col(ln_b)

    dec8_b = [atile(decoder_inputs[b], _F8NP) for b in range(B)]
    enc8_b = [atile(encoder_states[b], _F8NP) for b in range(B)]
    mt_b = [col(tgt_attention_mask[b, 0, 0, :]) for b in range(B)]
    ms_b = [col(src_attention_mask[b, 0, 0, :]) for b in range(B)]

    in_maps = []
    for c in range(8):
        b, half = c // 2, c % 2
        in_maps.append({
            "dec8": dec8_b[b],
            "enc8": enc8_b[b],
            "dqb": atile(decoder_inputs[b][half * R:(half + 1) * R], _BFNP),
            "wq": wq_t, "wk": wk_t, "wv": wv_t, "wd": wd_t,
            "bq": bq_, "bde": bde_,
            "lng": g_, "lnb": b_,
            "mt": mt_b[b], "ms": ms_b[b],
            "onesr": _ONESR,
        })
    return in_maps


def kernel(**inputs):
    global _NC
    if _NC is None:
        _NC = build_kernel()
    nc = _NC
    in_maps = make_in_maps(**inputs)
    res = run_bass_kernel_spmd(nc, in_maps, core_ids=list(range(8)))
    out = np.empty((B, T, H), np.float32)
    for c in range(8):
        b, half = c // 2, c % 2
        buf = res.results[c]["out"]  # [p, ot, t]
        out[b, half * R:(half + 1) * R, :] = (
            buf.transpose(2, 1, 0).reshape(R, H))
    return out


# revision 31
# speedup vs baseline: 1.1875x; 1.0300x over previous
"""Trainium2 Bass kernel for an ALBERT-style seq2seq block (self-attn + cross-attn).

Sharding: 8 cores = (batch b in 0..3) x (decoder-row half in 0..1); zero
inter-core communication. Each core computes its 512 decoder rows.

v2 design vs baseline:
- All activations/weights in bf16; k/v projections in fp8(e4m3) with
  DoubleRow perf-mode matmuls (2 k-subtiles per pass, 0.5 cyc/col).
- v computed directly in [keys, vdim] layout (stationary = activation
  chunk), eliminating all PE transposes.
- k bias dropped (softmax shift-invariant), v bias folded into the
  out-projection bias on the host (bd_eff = bd + bv @ Wd.T).
- PSUM evictions and softmax-denominator copies moved off the Scalar
  engine (exp is the Scalar bottleneck) to GpSimd/Vector.
- Leaner LN tail (Rsqrt fused); held fillers drain BEFORE the broadcast
  matmuls so the PE queue is never head-of-line blocked on LN stats.
"""

import sys

sys.path.insert(0, "/opt/trn_rl_repo")

import numpy as np
import ml_dtypes

import concourse.bacc as bacc
import concourse.mybir as mybir
from concourse.bass_utils import run_bass_kernel_spmd
from concourse.tile import TileContext

F32 = mybir.dt.float32
F32R = mybir.dt.float32r
BF16 = mybir.dt.bfloat16
F8 = mybir.dt.float8e4
AF = mybir.ActivationFunctionType
ALU = mybir.AluOpType
DR = mybir.MatmulPerfMode.DoubleRow

P = 128          # partitions
H = 1024         # hidden
NT = H // P      # 8 tiles over hidden
NH = 16          # heads
D = 64           # head dim
T = 1024         # sequence length (encoder and decoder)
R = 512          # decoder rows per core
B = 4
EPS = 1e-12
WS = 32.0        # fp8 weight pre-scale (host); descaled at PSUM eviction
INV = 1.0 / WS


def build_kernel():
    nc = bacc.Bacc("TRN2", num_devices=8)

    dec8_d = nc.declare_dram_parameter("dec8", [P, NT, T], F8, isOutput=False)
    enc8_d = nc.declare_dram_parameter("enc8", [P, NT, T], F8, isOutput=False)
    dqb_d = nc.declare_dram_parameter("dqb", [P, NT, R], BF16, isOutput=False)
    wq_d = nc.declare_dram_parameter("wq", [NT, P, NT, P], BF16, isOutput=False)
    wk_d = nc.declare_dram_parameter("wk", [P, NT, NT, P], F8, isOutput=False)
    wv_d = nc.declare_dram_parameter("wv", [P, NT, H], F8, isOutput=False)
    wd_d = nc.declare_dram_parameter("wd", [NT, P, NT, P], F8, isOutput=False)
    bq_d = nc.declare_dram_parameter("bq", [P, NT, 1], F32, isOutput=False)
    bde_d = nc.declare_dram_parameter("bde", [P, NT, 1], F32, isOutput=False)
    lng_d = nc.declare_dram_parameter("lng", [P, NT, 1], F32, isOutput=False)
    lnb_d = nc.declare_dram_parameter("lnb", [P, NT, 1], F32, isOutput=False)
    mt_d = nc.declare_dram_parameter("mt", [P, NT, 1], F32, isOutput=False)
    ms_d = nc.declare_dram_parameter("ms", [P, NT, 1], F32, isOutput=False)
    onesrr_d = nc.declare_dram_parameter("onesr", [1, P], F32, isOutput=False)
    out_d = nc.declare_dram_parameter("out", [P, NT, R], F32, isOutput=True)

    with TileContext(nc) as tc:
        with tc.tile_pool(name="base", bufs=1) as base:
            x8 = base.tile([P, NT, T], F8, tag="x8")
            x8e = base.tile([P, NT, T], F8, tag="x8e")
            dqb = base.tile([P, NT, R], BF16, tag="dqb")
            wk = base.tile([P, NT, NT, P], F8, tag="wk")
            wv = base.tile([P, NT, H], F8, tag="wv")
            kT = base.tile([P, NT, T], BF16, tag="kT")
            vi = base.tile([P, NT, NH, P], BF16, tag="vi")
            q1b = [base.tile([P, R], BF16, tag=f"q1b{o}", name=f"q1b{o}") for o in range(NT)]
            slfb = [base.tile([P, R], BF16, tag=f"slfb{o}", name=f"slfb{o}") for o in range(NT)]
            ctxn = base.tile([P, NT, R], F8, tag="ctxn")
            resT = base.tile([P, NT, R], BF16, tag="resT")
            partialA = base.tile([P, NT, R], F32, tag="partialA")
            wdh = base.tile([P, NT, 4, P], F8, tag="wdh")
            bqc = base.tile([P, NT, 1], F32, tag="bqc")
            bdec = base.tile([P, NT, 1], F32, tag="bdec")
            gc = base.tile([P, NT, 1], F32, tag="gc")
            bc = base.tile([P, NT, 1], F32, tag="bc")
            mtc = base.tile([P, NT, 1], F32, tag="mtc")
            msc = base.tile([P, NT, 1], F32, tag="msc")
            onesb = base.tile([P, 1], BF16, tag="onesb")
            onesr = base.tile([1, P], F32R, tag="onesr")
            epsc = base.tile([P, 1], F32, tag="epsc")

            # ---- startup: spread initial DMAs across engine queues ----
            nc.sync.dma_start(out=x8[:, 0:4, :], in_=dec8_d.ap()[:, 0:4, :])
            nc.scalar.dma_start(out=x8[:, 4:NT, :], in_=dec8_d.ap()[:, 4:NT, :])
            nc.gpsimd.dma_start(out=wk[:, :, :, :], in_=wk_d.ap())
            nc.sync.dma_start(out=wv[:, :, :], in_=wv_d.ap())
            nc.scalar.dma_start(out=dqb[:, :, :], in_=dqb_d.ap())
            nc.scalar.dma_start(out=x8e[:, :, :], in_=enc8_d.ap())
            nc.scalar.dma_start(
                out=wdh[:, :, :, :],
                in_=wd_d.ap().rearrange("o p i c -> p o i c")[:, :, 4:NT, :])
            nc.gpsimd.dma_start(out=bqc[:, :, :], in_=bq_d.ap())
            nc.gpsimd.dma_start(out=bdec[:, :, :], in_=bde_d.ap())
            nc.gpsimd.dma_start(out=gc[:, :, :], in_=lng_d.ap())
            nc.gpsimd.dma_start(out=bc[:, :, :], in_=lnb_d.ap())
            nc.gpsimd.dma_start(out=mtc[:, :, :], in_=mt_d.ap())
            nc.gpsimd.dma_start(out=msc[:, :, :], in_=ms_d.ap())
            nc.gpsimd.dma_start(out=onesr[:, :], in_=onesrr_d.ap().bitcast(F32R))
            nc.gpsimd.memset(onesb[:, :], 1.0)
            nc.vector.memset(epsc[:, :], EPS)
            # softmax-denominator ones rows of the PV stationary blocks
            for st in range(NT):
                nc.gpsimd.memset(vi[:, st, :, D:P], 1.0)

            prp = tc.alloc_tile_pool(name="prp", bufs=4)
            rcp = tc.alloc_tile_pool(name="rcp", bufs=3)

            def k_unit(src, ot, ps, uid):
                """One o-tile of a k projection: fp8 DoubleRow matmuls,
                descaled eviction into kT (no k bias needed)."""
                for tch in range(2):
                    tsl = slice(tch * R, (tch + 1) * R)
                    pk = ps.tile([P, R], F32, tag="pk", name=f"pk{uid}_{ot}_{tch}")
                    for i in range(4):
                        nc.tensor.matmul(
                            pk[:, :], wk[:, ot, 2 * i:2 * i + 2, :],
                            src[:, 2 * i:2 * i + 2, tsl],
                            start=(i == 0), stop=(i == 3), perf_mode=DR)
                    nc.vector.tensor_scalar_mul(kT[:, ot, tsl], pk[:, :], INV)

            def v_unit(src, kb, ps, uid):
                """One key-block of a v projection, computed directly in
                [keys, vdim] layout (stationary = fp8 activation chunk)."""
                ksl = slice(kb * P, (kb + 1) * P)
                for hf in range(2):
                    vsl = slice(hf * R, (hf + 1) * R)
                    pv = ps.tile([P, R], F32, tag="pk", name=f"pv{uid}_{kb}_{hf}")
                    for i in range(4):
                        nc.tensor.matmul(
                            pv[:, :], src[:, 2 * i:2 * i + 2, ksl],
                            wv[:, 2 * i:2 * i + 2, vsl],
                            start=(i == 0), stop=(i == 3), perf_mode=DR)
                    nc.scalar.activation(
                        vi[:, kb, hf * 8:(hf + 1) * 8, 0:D],
                        pv[:, :].rearrange("p (h c) -> p h c", c=D),
                        AF.Identity, scale=INV)

            def q_unit(ot, ps, wp):
                """One o-tile of the q projection (bf16, biased)."""
                wqc = wp.tile([P, NT, P], BF16, tag="wqc", name=f"wqc_{ot}")
                nc.gpsimd.dma_start(out=wqc[:, :, :], in_=wq_d.ap()[ot])
                pq = ps.tile([P, R], F32, tag="pk", name=f"pq_{ot}")
                for it in range(NT):
                    nc.tensor.matmul(
                        pq[:, :], wqc[:, it, :], dqb[:, it, :],
                        start=(it == 0), stop=(it == NT - 1))
                nc.scalar.activation(
                    q1b[ot][:, :], pq[:, :], AF.Identity, bias=bqc[:, ot, :])

            def attention(qsrc, mcol, fillers, uid, min_pair=0):
                """scoresT -> batched exp -> fused PV+denominator -> ctxn.
                fillers: callbacks emitting independent PE work, drained
                across pair iterations (only once j >= min_pair)."""
                fill_i = 0
                with tc.tile_pool(name="psc", bufs=2, space="PSUM") as psc, \
                     tc.tile_pool(name="pcx", bufs=1, space="PSUM") as pcx:
                    for j in range(NH // 2):
                        c0 = pcx.tile([P, R], F32, tag="c0", name=f"c0{uid}_{j}")
                        c1 = pcx.tile([P, R], F32, tag="c1", name=f"c1{uid}_{j}")
                        probs = [None] * NT
                        for st in range(NT + 1):
                            # scores + exp for step st; PV for step st-1 (SW
                            # pipeline so the PE never waits on the current exp)
                            if st < NT:
                                ssl = slice(st * P, (st + 1) * P)
                                s01 = psc.tile([P, 2, R], F32, tag="s01", name=f"s{uid}_{j}_{st}")
                                nc.tensor.matmul(
                                    s01[:, 0, :], kT[0:D, j, ssl], qsrc[j][0:D, :])
                                nc.tensor.matmul(
                                    s01[:, 1, :], kT[D:P, j, ssl], qsrc[j][D:P, :])
                                p01 = prp.tile([P, 2, R], BF16, tag="p01", name=f"p{uid}_{j}_{st}")
                                nc.scalar.activation(
                                    p01[:, :, :], s01[:, :, :], AF.Exp,
                                    bias=mcol[:, st, :], scale=0.125)
                                probs[st] = p01
                            if st > 0:
                                pp01 = probs[st - 1]
                                nc.tensor.matmul(
                                    c0[:, :], vi[:, st - 1, 2 * j, :], pp01[:, 0, :],
                                    start=(st == 1), stop=(st == NT))
                                nc.tensor.matmul(
                                    c1[:, :], vi[:, st - 1, 2 * j + 1, :], pp01[:, 1, :],
                                    start=(st == 1), stop=(st == NT))

                        cc0 = rcp.tile([P, R], F32, tag="cc", bufs=2, name=f"cc0{uid}_{j}")
                        cc1 = rcp.tile([P, R], F32, tag="cc", bufs=2, name=f"cc1{uid}_{j}")
                        # DVE copy frees the PSUM bank; keeps Scalar on exp
                        nc.vector.tensor_copy(cc0[:, :], c0[:, :])
                        nc.vector.tensor_copy(cc1[:, :], c1[:, :])
                        d0 = rcp.tile([D, R], F32, tag="rr", bufs=6, name=f"d0{uid}_{j}")
                        d1 = rcp.tile([D, R], F32, tag="rr", bufs=6, name=f"d1{uid}_{j}")
                        r0 = rcp.tile([D, R], F32, tag="rr", bufs=6, name=f"r0{uid}_{j}")
                        r1 = rcp.tile([D, R], F32, tag="rr", bufs=6, name=f"r1{uid}_{j}")
                        nc.vector.tensor_copy(d0[:, :], cc0[D:P, :])
                        nc.vector.reciprocal_approx_fast(r0[:, :], d0[:, :])
                        nc.vector.tensor_mul(ctxn[0:D, j, :], cc0[0:D, :], r0[:, :])
                        nc.vector.tensor_copy(d1[:, :], cc1[D:P, :])
                        nc.vector.reciprocal_approx_fast(r1[:, :], d1[:, :])
                        nc.vector.tensor_mul(ctxn[D:P, j, :], cc1[0:D, :], r1[:, :])
                        navail = NH // 2 - min_pair
                        while (fillers and j >= min_pair and
                               fill_i < (j - min_pair + 1) * len(fillers) // navail):
                            fillers[fill_i]()
                            fill_i += 1
                    while fill_i < len(fillers):
                        fillers[fill_i]()
                        fill_i += 1

            def proj_ln(resid_b, dst, fillers, uid, partial=None, hold=0,
                        store=False):
                """Out-projection + residual into resT with LN stats fused
                per o-tile; then row stats, broadcast, per-o-tile apply -> dst.
                If partial is given, it holds ht 0..3 of the accumulation and
                only ht 4..7 run here. Held fillers drain BEFORE the broadcast
                matmuls so the PE queue isn't blocked on the stats chain."""
                fill_i = 0
                with tc.tile_pool(name="wdp", bufs=2) as wdp, \
                     tc.tile_pool(name="sqp", bufs=2) as sqp, \
                     tc.tile_pool(name="lnp", bufs=1) as lnp, \
                     tc.tile_pool(name="outp", bufs=2) as outp, \
                     tc.tile_pool(name="ps3", bufs=2, space="PSUM") as ps, \
                     tc.tile_pool(name="ps4", bufs=1, space="PSUM") as ps4:
                    pmu = ps4.tile([1, R], F32, tag="pmu", name=f"pmu{uid}")
                    psq = ps4.tile([1, R], F32, tag="psq", name=f"psq{uid}")
                    for ot in range(NT):
                        pp = ps.tile([P, R], F32, tag="pp", name=f"pp{uid}_{ot}")
                        if partial is None:
                            wdc = wdp.tile([P, NT, P], F8, tag="wd", name=f"wd{uid}_{ot}")
                            nc.sync.dma_start(out=wdc[:, :, :], in_=wd_d.ap()[ot])
                            for i in range(4):
                                nc.tensor.matmul(
                                    pp[:, :], wdc[:, 2 * i:2 * i + 2, :],
                                    ctxn[:, 2 * i:2 * i + 2, :],
                                    start=(i == 0), stop=(i == 3), perf_mode=DR)
                            tmp = sqp.tile([P, R], F32, tag="ts", name=f"tmp{uid}_{ot}", bufs=2)
                            nc.vector.tensor_scalar(
                                tmp[:, :], pp[:, :], INV, bdec[:, ot, :],
                                op0=ALU.mult, op1=ALU.add)
                            nc.vector.tensor_add(
                                resT[:, ot, :], tmp[:, :], resid_b[ot][:, :])
                        else:
                            for i in range(2):
                                nc.tensor.matmul(
                                    pp[:, :], wdh[:, ot, 2 * i:2 * i + 2, :],
                                    ctxn[:, 2 * i + 4:2 * i + 6, :],
                                    start=(i == 0), stop=(i == 1), perf_mode=DR)
                            tsum = sqp.tile([P, R], F32, tag="ts", name=f"tsum{uid}_{ot}", bufs=2)
                            nc.vector.scalar_tensor_tensor(
                                tsum[:, :], pp[:, :], INV,
                                partial[:, ot, :], op0=ALU.mult, op1=ALU.add)
                            nc.vector.tensor_add(
                                resT[:, ot, :], tsum[:, :], resid_b[ot][:, :])
                        sq = sqp.tile([P, R], BF16, tag="sq", name=f"sq{uid}_{ot}", bufs=3)
                        nc.scalar.square(sq[:, :], resT[:, ot, :])
                        nc.tensor.matmul(
                            pmu[:, :], onesb[:, :], resT[:, ot, :],
                            start=(ot == 0), stop=(ot == NT - 1))
                        nc.tensor.matmul(
                            psq[:, :], onesb[:, :], sq[:, :],
                            start=(ot == 0), stop=(ot == NT - 1))
                        early = len(fillers) - hold
                        while fillers and fill_i < (ot + 1) * early // NT:
                            fillers[fill_i]()
                            fill_i += 1
                    # held fillers drain now: independent PE work queued ahead
                    # of the broadcast matmuls that wait on the stats chain
                    while fill_i < len(fillers):
                        fillers[fill_i]()
                        fill_i += 1
                    mu_r = lnp.tile([1, R], F32R, tag="lnrow", bufs=2, name=f"mu{uid}")
                    nc.scalar.mul(mu_r[:, :], pmu[:, :], 1.0 / H)
                    sq_r = lnp.tile([1, R], F32R, tag="lnrow", bufs=2, name=f"sqr{uid}")
                    nc.scalar.mul(sq_r[:, :], psq[:, :], 1.0 / H)
                    muB = ps4.tile([P, R], F32, tag="pmu", name=f"muBp{uid}")
                    nc.tensor.matmul(muB[:, :], onesr[:, :], mu_r[:, :])
                    sqBp = ps4.tile([P, R], F32, tag="psq", name=f"sqBp{uid}")
                    nc.tensor.matmul(sqBp[:, :], onesr[:, :], sq_r[:, :])
                    # free the PSUM banks fast: msB/muBb read muB, varB reads
                    # sqBp, then the whole apply runs from SBUF in bf16 (2x DVE)
                    msB = sqp.tile([P, R], F32, tag="lnB", name=f"msB{uid}", bufs=2)
                    nc.scalar.square(msB[:, :], muB[:, :])
                    varB = sqp.tile([P, R], F32, tag="lnB", name=f"varB{uid}", bufs=2)
                    nc.vector.tensor_sub(varB[:, :], sqBp[:, :], msB[:, :])
                    muBb = sqp.tile([P, R], BF16, tag="muBb", name=f"muBb{uid}", bufs=1)
                    nc.vector.tensor_copy(muBb[:, :], muB[:, :])
                    sdB = sqp.tile([P, R], F32, tag="lnB", name=f"sdB{uid}", bufs=2)
                    nc.scalar.activation(sdB[:, :], varB[:, :], AF.Sqrt, bias=epsc[:, :])
                    rsB = sqp.tile([P, R], F32, tag="rsB", name=f"rsB{uid}", bufs=1)
                    nc.vector.reciprocal_approx_fast(rsB[:, :], sdB[:, :])
                    rsBb = sqp.tile([P, R], BF16, tag="rsBb", name=f"rsBb{uid}", bufs=1)
                    nc.vector.tensor_copy(rsBb[:, :], rsB[:, :])
                    for ot in range(NT):
                        t1 = sqp.tile([P, R], BF16, tag="tt", name=f"t1{uid}_{ot}", bufs=4)
                        nc.vector.tensor_sub(t1[:, :], resT[:, ot, :], muBb[:, :])
                        t2 = sqp.tile([P, R], BF16, tag="tt", name=f"t2{uid}_{ot}", bufs=4)
                        nc.vector.tensor_mul(t2[:, :], t1[:, :], rsBb[:, :])
                        if store:
                            oT = outp.tile([P, R], F32, tag="oT", name=f"oT{uid}_{ot}")
                            nc.scalar.activation(
                                oT[:, :], t2[:, :], AF.Identity,
                                bias=bc[:, ot, :], scale=gc[:, ot, :])
                            nc.sync.dma_start(out=out_d.ap()[:, ot, :], in_=oT[:, :])
                        else:
                            nc.gpsimd.tensor_scalar(
                                dst[ot][:, :], t2[:, :], gc[:, ot, :], bc[:, ot, :],
                                op0=ALU.mult, op1=ALU.add)

            # ================== phase 1: decoder projections ==================
            # ===== + phase 2: self-attn (k/q tail and encoder-k as fills) =====
            with tc.tile_pool(name="wqp", bufs=3) as wqp, \
                 tc.tile_pool(name="psA", bufs=2, space="PSUM") as psA:
                k_unit(x8, 0, psA, "a")
                k_unit(x8, 1, psA, "a")
                for kb in range(NT):
                    v_unit(x8, kb, psA, "a")
                q_unit(0, psA, wqp)
                q_unit(1, psA, wqp)

                fills = []
                for ot in range(2, NT):
                    fills.append(lambda ot=ot: k_unit(x8, ot, psA, "a"))
                    fills.append(lambda ot=ot: q_unit(ot, psA, wqp))
                for ot in range(5):
                    fills.append(lambda ot=ot: k_unit(x8e, ot, psA, "b"))
                attention(q1b, mtc, fills, "A")

            # ========= phase 3: out-proj + LN1 (+ encoder-v interleaved) =====
            with tc.tile_pool(name="psV", bufs=2, space="PSUM") as psV:
                evs = [lambda kb=kb: v_unit(x8e, kb, psV, "b") for kb in range(NT)]
                proj_ln(q1b, slfb, evs, "A", hold=4)

            # ==================== phase 4: cross-attention ====================
            with tc.tile_pool(name="wdpB", bufs=2) as wdpB, \
                 tc.tile_pool(name="psB", bufs=2, space="PSUM") as psB:
                def mk_pA(ot):
                    def f():
                        wdc = wdpB.tile([P, 4, P], F8, tag="wdA", name=f"wdA{ot}")
                        nc.sync.dma_start(out=wdc[:, :, :], in_=wd_d.ap()[ot][:, 0:4, :])
                        pp = psB.tile([P, R], F32, tag="pk", name=f"ppA{ot}")
                        for i in range(2):
                            nc.tensor.matmul(
                                pp[:, :], wdc[:, 2 * i:2 * i + 2, :],
                                ctxn[:, 2 * i:2 * i + 2, :],
                                start=(i == 0), stop=(i == 1), perf_mode=DR)
                        nc.vector.tensor_scalar(
                            partialA[:, ot, :], pp[:, :], INV, bdec[:, ot, :],
                            op0=ALU.mult, op1=ALU.add)
                    return f

                fillsB = [lambda ot=ot: k_unit(x8e, ot, psB, "b") for ot in range(5, NT)]
                fillsB += [mk_pA(ot) for ot in range(NT)]
                attention(slfb, msc, fillsB, "B")

            # ============ phase 5: out-proj + LN2 + store ====================
            proj_ln(slfb, None, [], "B", partial=partialA, store=True)
            rcp.release()
            prp.release()

    nc.compile()
    return nc


_NC = None

_F8NP = ml_dtypes.float8_e4m3
_BFNP = ml_dtypes.bfloat16
_ONESR = np.ones((1, P), np.float32)


def make_in_maps(encoder_states, decoder_inputs, src_attention_mask,
                 tgt_attention_mask, Wq, bq, Wk, bk, Wv, bv, Wd, bd, ln_g, ln_b):
    f = np.float32

    def wtile(w, dt, scale=1.0):  # [o,i] -> W.T chunks [ot, p_i, it, p_o]
        a = (np.asarray(w, f).T * scale).reshape(NT, P, NT, P)
        return np.ascontiguousarray(a.transpose(2, 1, 0, 3)).astype(dt)

    def atile(x, dt):  # [t,i] -> x.T tiled [p, it, t]
        return np.ascontiguousarray(
            np.asarray(x, f).T.reshape(NT, P, -1).transpose(1, 0, 2)).astype(dt)

    col = lambda x: np.ascontiguousarray(
        np.asarray(x, f).reshape(NT, P).T.reshape(P, NT, 1))

    wq_t = wtile(Wq, _BFNP)
    wd_t = wtile(Wd, _F8NP, WS)
    # wk resident layout [p_i, ot, it, p_o]
    wk_t = np.ascontiguousarray(
        wtile(Wk, np.float32, WS).transpose(1, 0, 2, 3)).astype(_F8NP)
    # wv v-direct layout [p_i, it, o]
    wv_t = np.ascontiguousarray(
        (np.asarray(Wv, f).T * WS).reshape(NT, P, H).transpose(1, 0, 2)
    ).astype(_F8NP)
    bde = np.asarray(bd, f) + np.asarray(bv, f) @ np.asarray(Wd, f).T
    bq_, bde_ = col(bq), col(bde)
    g_, b_ = col(ln_g), col(ln_b)

    dec8_b = [atile(decoder_inputs[b], _F8NP) for b in range(B)]
    enc8_b = [atile(encoder_states[b], _F8NP) for b in range(B)]
    mt_b = [col(tgt_attention_mask[b, 0, 0, :]) for b in range(B)]
    ms_b = [col(src_attention_mask[b, 0, 0, :]) for b in range(B)]

    in_maps = []
    for c in range(8):
        b, half = c // 2, c % 2
        in_maps.append({
            "dec8": dec8_b[b],
            "enc8": enc8_b[b],
            "dqb": atile(decoder_inputs[b][half * R:(half + 1) * R], _BFNP),
            "wq": wq_t, "wk": wk_t, "wv": wv_t, "wd": wd_t,
            "bq": bq_, "bde": bde_,
            "lng": g_, "lnb": b_,
            "mt": mt_b[b], "ms": ms_b[b],
            "onesr": _ONESR,
        })
    return in_maps


def kernel(**inputs):
    global _NC
    if _NC is None:
        _NC = build_kernel()
    nc = _NC
    in_maps = make_in_maps(**inputs)
    res = run_bass_kernel_spmd(nc, in_maps, core_ids=list(range(8)))
    out = np.empty((B, T, H), np.float32)
    for c in range(8):
        b, half = c // 2, c % 2
        buf = res.results[c]["out"]  # [p, ot, t]
        out[b, half * R:(half + 1) * R, :] = (
            buf.transpose(2, 1, 0).reshape(R, H))
    return out


# revision 32
# speedup vs baseline: 1.2049x; 1.0146x over previous
"""Trainium2 Bass kernel for an ALBERT-style seq2seq block (self-attn + cross-attn).

Sharding: 8 cores = (batch b in 0..3) x (decoder-row half in 0..1); zero
inter-core communication. Each core computes its 512 decoder rows.

v2 design vs baseline:
- All activations/weights in bf16; k/v projections in fp8(e4m3) with
  DoubleRow perf-mode matmuls (2 k-subtiles per pass, 0.5 cyc/col).
- v computed directly in [keys, vdim] layout (stationary = activation
  chunk), eliminating all PE transposes.
- k bias dropped (softmax shift-invariant), v bias folded into the
  out-projection bias on the host (bd_eff = bd + bv @ Wd.T).
- PSUM evictions and softmax-denominator copies moved off the Scalar
  engine (exp is the Scalar bottleneck) to GpSimd/Vector.
- Leaner LN tail (Rsqrt fused); held fillers drain BEFORE the broadcast
  matmuls so the PE queue is never head-of-line blocked on LN stats.
"""

import sys

sys.path.insert(0, "/opt/trn_rl_repo")

import numpy as np
import ml_dtypes

import concourse.bacc as bacc
import concourse.mybir as mybir
from concourse.bass_utils import run_bass_kernel_spmd
from concourse.tile import TileContext

F32 = mybir.dt.float32
F32R = mybir.dt.float32r
BF16 = mybir.dt.bfloat16
F8 = mybir.dt.float8e4
AF = mybir.ActivationFunctionType
ALU = mybir.AluOpType
DR = mybir.MatmulPerfMode.DoubleRow

P = 128          # partitions
H = 1024         # hidden
NT = H // P      # 8 tiles over hidden
NH = 16          # heads
D = 64           # head dim
T = 1024         # sequence length (encoder and decoder)
R = 512          # decoder rows per core
B = 4
EPS = 1e-12
WS = 32.0        # fp8 weight pre-scale (host); descaled at PSUM eviction
INV = 1.0 / WS


def build_kernel():
    nc = bacc.Bacc("TRN2", num_devices=8)

    dec8_d = nc.declare_dram_parameter("dec8", [P, NT, T], F8, isOutput=False)
    enc8_d = nc.declare_dram_parameter("enc8", [P, NT, T], F8, isOutput=False)
    dqb_d = nc.declare_dram_parameter("dqb", [P, NT, R], BF16, isOutput=False)
    wq_d = nc.declare_dram_parameter("wq", [NT, P, NT, P], BF16, isOutput=False)
    wk_d = nc.declare_dram_parameter("wk", [P, NT, NT, P], F8, isOutput=False)
    wv_d = nc.declare_dram_parameter("wv", [P, NT, H], F8, isOutput=False)
    wd_d = nc.declare_dram_parameter("wd", [NT, P, NT, P], F8, isOutput=False)
    bq_d = nc.declare_dram_parameter("bq", [P, NT, 1], F32, isOutput=False)
    bde_d = nc.declare_dram_parameter("bde", [P, NT, 1], F32, isOutput=False)
    lng_d = nc.declare_dram_parameter("lng", [P, NT, 1], F32, isOutput=False)
    lnb_d = nc.declare_dram_parameter("lnb", [P, NT, 1], F32, isOutput=False)
    mt_d = nc.declare_dram_parameter("mt", [P, NT, 1], F32, isOutput=False)
    ms_d = nc.declare_dram_parameter("ms", [P, NT, 1], F32, isOutput=False)
    onesrr_d = nc.declare_dram_parameter("onesr", [1, P], F32, isOutput=False)
    out_d = nc.declare_dram_parameter("out", [P, NT, R], F32, isOutput=True)

    with TileContext(nc) as tc:
        with tc.tile_pool(name="base", bufs=1) as base:
            x8 = base.tile([P, NT, T], F8, tag="x8")
            x8e = base.tile([P, NT, T], F8, tag="x8e")
            dqb = base.tile([P, NT, R], BF16, tag="dqb")
            wk = base.tile([P, NT, NT, P], F8, tag="wk")
            wv = base.tile([P, NT, H], F8, tag="wv")
            kT = base.tile([P, NT, T], BF16, tag="kT")
            vi = base.tile([P, NT, NH, P], BF16, tag="vi")
            q1b = [base.tile([P, R], BF16, tag=f"q1b{o}", name=f"q1b{o}") for o in range(NT)]
            slfb = [base.tile([P, R], BF16, tag=f"slfb{o}", name=f"slfb{o}") for o in range(NT)]
            ctxn = base.tile([P, NT, R], F8, tag="ctxn")
            resT = base.tile([P, NT, R], BF16, tag="resT")
            partialA = base.tile([P, NT, R], F32, tag="partialA")
            wdh = base.tile([P, NT, 4, P], F8, tag="wdh")
            bqc = base.tile([P, NT, 1], F32, tag="bqc")
            bdec = base.tile([P, NT, 1], F32, tag="bdec")
            gc = base.tile([P, NT, 1], F32, tag="gc")
            bc = base.tile([P, NT, 1], F32, tag="bc")
            mtc = base.tile([P, NT, 1], F32, tag="mtc")
            msc = base.tile([P, NT, 1], F32, tag="msc")
            onesb = base.tile([P, 1], BF16, tag="onesb")
            onesr = base.tile([1, P], F32R, tag="onesr")
            epsc = base.tile([P, 1], F32, tag="epsc")

            # ---- startup: spread initial DMAs across engine queues ----
            nc.sync.dma_start(out=x8[:, 0:4, :], in_=dec8_d.ap()[:, 0:4, :])
            nc.scalar.dma_start(out=x8[:, 4:NT, :], in_=dec8_d.ap()[:, 4:NT, :])
            nc.gpsimd.dma_start(out=wk[:, :, :, :], in_=wk_d.ap())
            nc.sync.dma_start(out=wv[:, :, :], in_=wv_d.ap())
            nc.scalar.dma_start(out=dqb[:, :, :], in_=dqb_d.ap())
            nc.scalar.dma_start(out=x8e[:, :, :], in_=enc8_d.ap())
            nc.scalar.dma_start(
                out=wdh[:, :, :, :],
                in_=wd_d.ap().rearrange("o p i c -> p o i c")[:, :, 4:NT, :])
            nc.gpsimd.dma_start(out=bqc[:, :, :], in_=bq_d.ap())
            nc.gpsimd.dma_start(out=bdec[:, :, :], in_=bde_d.ap())
            nc.gpsimd.dma_start(out=gc[:, :, :], in_=lng_d.ap())
            nc.gpsimd.dma_start(out=bc[:, :, :], in_=lnb_d.ap())
            nc.gpsimd.dma_start(out=mtc[:, :, :], in_=mt_d.ap())
            nc.gpsimd.dma_start(out=msc[:, :, :], in_=ms_d.ap())
            nc.gpsimd.dma_start(out=onesr[:, :], in_=onesrr_d.ap().bitcast(F32R))
            nc.gpsimd.memset(onesb[:, :], 1.0)
            nc.vector.memset(epsc[:, :], EPS)
            # softmax-denominator ones rows of the PV stationary blocks
            for st in range(NT):
                nc.gpsimd.memset(vi[:, st, :, D:P], 1.0)

            prp = tc.alloc_tile_pool(name="prp", bufs=4)
            rcp = tc.alloc_tile_pool(name="rcp", bufs=3)

            def k_unit(src, ot, ps, uid):
                """One o-tile of a k projection: fp8 DoubleRow matmuls,
                descaled eviction into kT (no k bias needed)."""
                for tch in range(2):
                    tsl = slice(tch * R, (tch + 1) * R)
                    pk = ps.tile([P, R], F32, tag="pk", name=f"pk{uid}_{ot}_{tch}")
                    for i in range(4):
                        nc.tensor.matmul(
                            pk[:, :], wk[:, ot, 2 * i:2 * i + 2, :],
                            src[:, 2 * i:2 * i + 2, tsl],
                            start=(i == 0), stop=(i == 3), perf_mode=DR)
                    nc.vector.tensor_scalar_mul(kT[:, ot, tsl], pk[:, :], INV)

            def v_unit(src, kb, ps, uid):
                """One key-block of a v projection, computed directly in
                [keys, vdim] layout (stationary = fp8 activation chunk)."""
                ksl = slice(kb * P, (kb + 1) * P)
                for hf in range(2):
                    vsl = slice(hf * R, (hf + 1) * R)
                    pv = ps.tile([P, R], F32, tag="pk", name=f"pv{uid}_{kb}_{hf}")
                    for i in range(4):
                        nc.tensor.matmul(
                            pv[:, :], src[:, 2 * i:2 * i + 2, ksl],
                            wv[:, 2 * i:2 * i + 2, vsl],
                            start=(i == 0), stop=(i == 3), perf_mode=DR)
                    nc.scalar.activation(
                        vi[:, kb, hf * 8:(hf + 1) * 8, 0:D],
                        pv[:, :].rearrange("p (h c) -> p h c", c=D),
                        AF.Identity, scale=INV)

            def q_unit(ot, ps, wp):
                """One o-tile of the q projection (bf16, biased)."""
                wqc = wp.tile([P, NT, P], BF16, tag="wqc", name=f"wqc_{ot}")
                nc.gpsimd.dma_start(out=wqc[:, :, :], in_=wq_d.ap()[ot])
                pq = ps.tile([P, R], F32, tag="pk", name=f"pq_{ot}")
                for it in range(NT):
                    nc.tensor.matmul(
                        pq[:, :], wqc[:, it, :], dqb[:, it, :],
                        start=(it == 0), stop=(it == NT - 1))
                nc.scalar.activation(
                    q1b[ot][:, :], pq[:, :], AF.Identity, bias=bqc[:, ot, :])

            def attention(qsrc, mcol, fillers, uid, min_pair=0):
                """scoresT -> batched exp -> fused PV+denominator -> ctxn.
                fillers: callbacks emitting independent PE work, drained
                across pair iterations (only once j >= min_pair)."""
                fill_i = 0
                with tc.tile_pool(name="psc", bufs=2, space="PSUM") as psc, \
                     tc.tile_pool(name="pcx", bufs=1, space="PSUM") as pcx:
                    pending = {}

                    def emit_scores(idx):
                        j2, st2 = divmod(idx, NT)
                        ssl = slice(st2 * P, (st2 + 1) * P)
                        s01 = psc.tile([P, 2, R], F32, tag="s01", name=f"s{uid}_{j2}_{st2}")
                        nc.tensor.matmul(
                            s01[:, 0, :], kT[0:D, j2, ssl], qsrc[j2][0:D, :])
                        nc.tensor.matmul(
                            s01[:, 1, :], kT[D:P, j2, ssl], qsrc[j2][D:P, :])
                        p01 = prp.tile([P, 2, R], BF16, tag="p01", name=f"p{uid}_{j2}_{st2}")
                        nc.scalar.activation(
                            p01[:, :, :], s01[:, :, :], AF.Exp,
                            bias=mcol[:, st2, :], scale=0.125)
                        pending[idx] = p01

                    # scores/exp run 2 steps ahead of PV ACROSS pairs, so the
                    # exp stream never stalls at a pair boundary
                    emit_scores(0)
                    emit_scores(1)
                    for j in range(NH // 2):
                        c0 = pcx.tile([P, R], F32, tag="c0", name=f"c0{uid}_{j}")
                        c1 = pcx.tile([P, R], F32, tag="c1", name=f"c1{uid}_{j}")
                        for st in range(NT):
                            nxt = j * NT + st + 2
                            if nxt < NT * (NH // 2):
                                emit_scores(nxt)
                            pp01 = pending.pop(j * NT + st)
                            nc.tensor.matmul(
                                c0[:, :], vi[:, st, 2 * j, :], pp01[:, 0, :],
                                start=(st == 0), stop=(st == NT - 1))
                            nc.tensor.matmul(
                                c1[:, :], vi[:, st, 2 * j + 1, :], pp01[:, 1, :],
                                start=(st == 0), stop=(st == NT - 1))

                        cc0 = rcp.tile([P, R], F32, tag="cc", bufs=2, name=f"cc0{uid}_{j}")
                        cc1 = rcp.tile([P, R], F32, tag="cc", bufs=2, name=f"cc1{uid}_{j}")
                        # DVE copy frees the PSUM bank; keeps Scalar on exp
                        nc.vector.tensor_copy(cc0[:, :], c0[:, :])
                        nc.vector.tensor_copy(cc1[:, :], c1[:, :])
                        d0 = rcp.tile([D, R], F32, tag="rr", bufs=6, name=f"d0{uid}_{j}")
                        d1 = rcp.tile([D, R], F32, tag="rr", bufs=6, name=f"d1{uid}_{j}")
                        r0 = rcp.tile([D, R], F32, tag="rr", bufs=6, name=f"r0{uid}_{j}")
                        r1 = rcp.tile([D, R], F32, tag="rr", bufs=6, name=f"r1{uid}_{j}")
                        nc.vector.tensor_copy(d0[:, :], cc0[D:P, :])
                        nc.vector.reciprocal_approx_fast(r0[:, :], d0[:, :])
                        nc.vector.tensor_mul(ctxn[0:D, j, :], cc0[0:D, :], r0[:, :])
                        nc.vector.tensor_copy(d1[:, :], cc1[D:P, :])
                        nc.vector.reciprocal_approx_fast(r1[:, :], d1[:, :])
                        nc.vector.tensor_mul(ctxn[D:P, j, :], cc1[0:D, :], r1[:, :])
                        navail = NH // 2 - min_pair
                        while (fillers and j >= min_pair and
                               fill_i < (j - min_pair + 1) * len(fillers) // navail):
                            fillers[fill_i]()
                            fill_i += 1
                    while fill_i < len(fillers):
                        fillers[fill_i]()
                        fill_i += 1

            def proj_ln(resid_b, dst, fillers, uid, partial=None, hold=0,
                        store=False):
                """Out-projection + residual into resT with LN stats fused
                per o-tile; then row stats, broadcast, per-o-tile apply -> dst.
                If partial is given, it holds ht 0..3 of the accumulation and
                only ht 4..7 run here. Held fillers drain BEFORE the broadcast
                matmuls so the PE queue isn't blocked on the stats chain."""
                fill_i = 0
                with tc.tile_pool(name="wdp", bufs=2) as wdp, \
                     tc.tile_pool(name="sqp", bufs=2) as sqp, \
                     tc.tile_pool(name="lnp", bufs=1) as lnp, \
                     tc.tile_pool(name="outp", bufs=2) as outp, \
                     tc.tile_pool(name="ps3", bufs=2, space="PSUM") as ps, \
                     tc.tile_pool(name="ps4", bufs=1, space="PSUM") as ps4:
                    pmu = ps4.tile([1, R], F32, tag="pmu", name=f"pmu{uid}")
                    psq = ps4.tile([1, R], F32, tag="psq", name=f"psq{uid}")
                    for ot in range(NT):
                        pp = ps.tile([P, R], F32, tag="pp", name=f"pp{uid}_{ot}")
                        if partial is None:
                            wdc = wdp.tile([P, NT, P], F8, tag="wd", name=f"wd{uid}_{ot}")
                            nc.sync.dma_start(out=wdc[:, :, :], in_=wd_d.ap()[ot])
                            for i in range(4):
                                nc.tensor.matmul(
                                    pp[:, :], wdc[:, 2 * i:2 * i + 2, :],
                                    ctxn[:, 2 * i:2 * i + 2, :],
                                    start=(i == 0), stop=(i == 3), perf_mode=DR)
                            tmp = sqp.tile([P, R], F32, tag="ts", name=f"tmp{uid}_{ot}", bufs=2)
                            nc.vector.tensor_scalar(
                                tmp[:, :], pp[:, :], INV, bdec[:, ot, :],
                                op0=ALU.mult, op1=ALU.add)
                            nc.vector.tensor_add(
                                resT[:, ot, :], tmp[:, :], resid_b[ot][:, :])
                        else:
                            for i in range(2):
                                nc.tensor.matmul(
                                    pp[:, :], wdh[:, ot, 2 * i:2 * i + 2, :],
                                    ctxn[:, 2 * i + 4:2 * i + 6, :],
                                    start=(i == 0), stop=(i == 1), perf_mode=DR)
                            tsum = sqp.tile([P, R], F32, tag="ts", name=f"tsum{uid}_{ot}", bufs=2)
                            nc.vector.scalar_tensor_tensor(
                                tsum[:, :], pp[:, :], INV,
                                partial[:, ot, :], op0=ALU.mult, op1=ALU.add)
                            nc.vector.tensor_add(
                                resT[:, ot, :], tsum[:, :], resid_b[ot][:, :])
                        sq = sqp.tile([P, R], BF16, tag="sq", name=f"sq{uid}_{ot}", bufs=3)
                        nc.scalar.square(sq[:, :], resT[:, ot, :])
                        nc.tensor.matmul(
                            pmu[:, :], onesb[:, :], resT[:, ot, :],
                            start=(ot == 0), stop=(ot == NT - 1))
                        nc.tensor.matmul(
                            psq[:, :], onesb[:, :], sq[:, :],
                            start=(ot == 0), stop=(ot == NT - 1))
                        early = len(fillers) - hold
                        while fillers and fill_i < (ot + 1) * early // NT:
                            fillers[fill_i]()
                            fill_i += 1
                    # held fillers drain now: independent PE work queued ahead
                    # of the broadcast matmuls that wait on the stats chain
                    while fill_i < len(fillers):
                        fillers[fill_i]()
                        fill_i += 1
                    mu_r = lnp.tile([1, R], F32R, tag="lnrow", bufs=2, name=f"mu{uid}")
                    nc.scalar.mul(mu_r[:, :], pmu[:, :], 1.0 / H)
                    sq_r = lnp.tile([1, R], F32R, tag="lnrow", bufs=2, name=f"sqr{uid}")
                    nc.scalar.mul(sq_r[:, :], psq[:, :], 1.0 / H)
                    muB = ps4.tile([P, R], F32, tag="pmu", name=f"muBp{uid}")
                    nc.tensor.matmul(muB[:, :], onesr[:, :], mu_r[:, :])
                    sqBp = ps4.tile([P, R], F32, tag="psq", name=f"sqBp{uid}")
                    nc.tensor.matmul(sqBp[:, :], onesr[:, :], sq_r[:, :])
                    # free the PSUM banks fast: msB/muBb read muB, varB reads
                    # sqBp, then the whole apply runs from SBUF in bf16 (2x DVE)
                    msB = sqp.tile([P, R], F32, tag="lnB", name=f"msB{uid}", bufs=2)
                    nc.scalar.square(msB[:, :], muB[:, :])
                    varB = sqp.tile([P, R], F32, tag="lnB", name=f"varB{uid}", bufs=2)
                    nc.vector.tensor_sub(varB[:, :], sqBp[:, :], msB[:, :])
                    muBb = sqp.tile([P, R], BF16, tag="muBb", name=f"muBb{uid}", bufs=1)
                    nc.vector.tensor_copy(muBb[:, :], muB[:, :])
                    sdB = sqp.tile([P, R], F32, tag="lnB", name=f"sdB{uid}", bufs=2)
                    nc.scalar.activation(sdB[:, :], varB[:, :], AF.Sqrt, bias=epsc[:, :])
                    rsB = sqp.tile([P, R], F32, tag="rsB", name=f"rsB{uid}", bufs=1)
                    nc.vector.reciprocal_approx_fast(rsB[:, :], sdB[:, :])
                    rsBb = sqp.tile([P, R], BF16, tag="rsBb", name=f"rsBb{uid}", bufs=1)
                    nc.vector.tensor_copy(rsBb[:, :], rsB[:, :])
                    for ot in range(NT):
                        t1 = sqp.tile([P, R], BF16, tag="tt", name=f"t1{uid}_{ot}", bufs=4)
                        nc.vector.tensor_sub(t1[:, :], resT[:, ot, :], muBb[:, :])
                        t2 = sqp.tile([P, R], BF16, tag="tt", name=f"t2{uid}_{ot}", bufs=4)
                        nc.vector.tensor_mul(t2[:, :], t1[:, :], rsBb[:, :])
                        if store:
                            oT = outp.tile([P, R], F32, tag="oT", name=f"oT{uid}_{ot}")
                            nc.scalar.activation(
                                oT[:, :], t2[:, :], AF.Identity,
                                bias=bc[:, ot, :], scale=gc[:, ot, :])
                            nc.sync.dma_start(out=out_d.ap()[:, ot, :], in_=oT[:, :])
                        else:
                            nc.gpsimd.tensor_scalar(
                                dst[ot][:, :], t2[:, :], gc[:, ot, :], bc[:, ot, :],
                                op0=ALU.mult, op1=ALU.add)

            # ================== phase 1: decoder projections ==================
            # ===== + phase 2: self-attn (k/q tail and encoder-k as fills) =====
            with tc.tile_pool(name="wqp", bufs=3) as wqp, \
                 tc.tile_pool(name="psA", bufs=2, space="PSUM") as psA:
                k_unit(x8, 0, psA, "a")
                k_unit(x8, 1, psA, "a")
                for kb in range(NT):
                    v_unit(x8, kb, psA, "a")
                q_unit(0, psA, wqp)
                q_unit(1, psA, wqp)

                fills = []
                for ot in range(2, NT):
                    fills.append(lambda ot=ot: k_unit(x8, ot, psA, "a"))
                    fills.append(lambda ot=ot: q_unit(ot, psA, wqp))
                for ot in range(5):
                    fills.append(lambda ot=ot: k_unit(x8e, ot, psA, "b"))
                attention(q1b, mtc, fills, "A")

            # ========= phase 3: out-proj + LN1 (+ encoder-v interleaved) =====
            with tc.tile_pool(name="psV", bufs=2, space="PSUM") as psV:
                evs = [lambda kb=kb: v_unit(x8e, kb, psV, "b") for kb in range(NT)]
                proj_ln(q1b, slfb, evs, "A", hold=4)

            # ==================== phase 4: cross-attention ====================
            with tc.tile_pool(name="wdpB", bufs=2) as wdpB, \
                 tc.tile_pool(name="psB", bufs=2, space="PSUM") as psB:
                def mk_pA(ot):
                    def f():
                        wdc = wdpB.tile([P, 4, P], F8, tag="wdA", name=f"wdA{ot}")
                        nc.sync.dma_start(out=wdc[:, :, :], in_=wd_d.ap()[ot][:, 0:4, :])
                        pp = psB.tile([P, R], F32, tag="pk", name=f"ppA{ot}")
                        for i in range(2):
                            nc.tensor.matmul(
                                pp[:, :], wdc[:, 2 * i:2 * i + 2, :],
                                ctxn[:, 2 * i:2 * i + 2, :],
                                start=(i == 0), stop=(i == 1), perf_mode=DR)
                        nc.vector.tensor_scalar(
                            partialA[:, ot, :], pp[:, :], INV, bdec[:, ot, :],
                            op0=ALU.mult, op1=ALU.add)
                    return f

                fillsB = [lambda ot=ot: k_unit(x8e, ot, psB, "b") for ot in range(5, NT)]
                fillsB += [mk_pA(ot) for ot in range(NT)]
                attention(slfb, msc, fillsB, "B")

            # ============ phase 5: out-proj + LN2 + store ====================
            proj_ln(slfb, None, [], "B", partial=partialA, store=True)
            rcp.release()
            prp.release()

    nc.compile()
    return nc


_NC = None

_F8NP = ml_dtypes.float8_e4m3
_BFNP = ml_dtypes.bfloat16
_ONESR = np.ones((1, P), np.float32)


def make_in_maps(encoder_states, decoder_inputs, src_attention_mask,
                 tgt_attention_mask, Wq, bq, Wk, bk, Wv, bv, Wd, bd, ln_g, ln_b):
    f = np.float32

    def wtile(w, dt, scale=1.0):  # [o,i] -> W.T chunks [ot, p_i, it, p_o]
        a = (np.asarray(w, f).T * scale).reshape(NT, P, NT, P)
        return np.ascontiguousarray(a.transpose(2, 1, 0, 3)).astype(dt)

    def atile(x, dt):  # [t,i] -> x.T tiled [p, it, t]
        return np.ascontiguousarray(
            np.asarray(x, f).T.reshape(NT, P, -1).transpose(1, 0, 2)).astype(dt)

    col = lambda x: np.ascontiguousarray(
        np.asarray(x, f).reshape(NT, P).T.reshape(P, NT, 1))

    wq_t = wtile(Wq, _BFNP)
    wd_t = wtile(Wd, _F8NP, WS)
    # wk resident layout [p_i, ot, it, p_o]
    wk_t = np.ascontiguousarray(
        wtile(Wk, np.float32, WS).transpose(1, 0, 2, 3)).astype(_F8NP)
    # wv v-direct layout [p_i, it, o]
    wv_t = np.ascontiguousarray(
        (np.asarray(Wv, f).T * WS).reshape(NT, P, H).transpose(1, 0, 2)
    ).astype(_F8NP)
    bde = np.asarray(bd, f) + np.asarray(bv, f) @ np.asarray(Wd, f).T
    bq_, bde_ = col(bq), col(bde)
    g_, b_ = col(ln_g), col(ln_b)

    dec8_b = [atile(decoder_inputs[b], _F8NP) for b in range(B)]
    enc8_b = [atile(encoder_states[b], _F8NP) for b in range(B)]
    mt_b = [col(tgt_attention_mask[b, 0, 0, :]) for b in range(B)]
    ms_b = [col(src_attention_mask[b, 0, 0, :]) for b in range(B)]

    in_maps = []
    for c in range(8):
        b, half = c // 2, c % 2
        in_maps.append({
            "dec8": dec8_b[b],
            "enc8": enc8_b[b],
            "dqb": atile(decoder_inputs[b][half * R:(half + 1) * R], _BFNP),
            "wq": wq_t, "wk": wk_t, "wv": wv_t, "wd": wd_t,
            "bq": bq_, "bde": bde_,
            "lng": g_, "lnb": b_,
            "mt": mt_b[b], "ms": ms_b[b],
            "onesr": _ONESR,
        })
    return in_maps


def kernel(**inputs):
    global _NC
    if _NC is None:
        _NC = build_kernel()
    nc = _NC
    in_maps = make_in_maps(**inputs)
    res = run_bass_kernel_spmd(nc, in_maps, core_ids=list(range(8)))
    out = np.empty((B, T, H), np.float32)
    for c in range(8):
        b, half = c // 2, c % 2
        buf = res.results[c]["out"]  # [p, ot, t]
        out[b, half * R:(half + 1) * R, :] = (
            buf.transpose(2, 1, 0).reshape(R, H))
    return out
